# revision 5
# baseline (speedup 1.0000x reference)
"""Trainium2 Bass kernel for nn_DBLoss (YOLO-style detection loss).

Strategy (pure data parallel over batch, 8 cores x 4 images):
  * The loss decomposes as 7.5*l_box + l_obj + 0.5*l_cls where only the
    objectness term touches every grid cell; box/cls terms only touch the
    ~180 label-assigned cells per image.
  * Host (numpy) replicates the reference's target assignment on the tiny
    `labels` tensor to produce per-core scatter metadata: positive-cell
    indices, gt-box constants, multi-hot class targets.  Collision
    semantics match the reference scatter: tbox last-write-wins, tcls
    accumulates classes (class is part of the scatter index).
  * Device: streams the p_raw shard to compute sum(focal_bce(obj_logit, 0))
    over all cells, gathers positive cells by indirect DMA, computes the
    obj t=1 correction, CIoU box loss and weighted focal cls loss there,
    and emits per-core partial sums.
  * Host sums 8x16 partials and applies the n_pos / mean normalizations.

All transcendentals use only the Exp and Ln ACT LUTs (one table set:
natural_log_exp_and_others), so a single act-table load suffices:
  softplus(x)        = ln(1 + exp(x))            (clamped at 88)
  sigmoid(x)^1.5     = exp(-1.5 * softplus(-x))
  (1-sigmoid(x))^1.5 = exp(-1.5 * softplus(x))
  sigmoid(x)         = 1/(1 + exp(-x))           (DVE reciprocal is exact)
  u^1.5              = exp(1.5 * ln(max(u, tiny)))
  arctan             = odd polynomial in z^2 after range reduction (DVE)
"""

import sys

sys.path.insert(0, "/opt/trn_rl_repo")

import numpy as np

import concourse.bass as bass
import concourse.tile as tile
from concourse import mybir
from concourse.bass import IndirectOffsetOnAxis
from concourse.bass_utils import run_bass_kernel_spmd

f32 = mybir.dt.float32
i32 = mybir.dt.int32
AF = mybir.ActivationFunctionType
ALU = mybir.AluOpType
AX = mybir.AxisListType

# problem constants (hardcoded per harness contract)
B, NA, H, W, M, C = 32, 3, 80, 80, 20, 80
CH = 5 + C
NCORES = 8
BL = B // NCORES            # 4 images per core
NCELL = BL * NA * H * W     # 76800 cells per core
NGRP = 6                    # positive-cell capacity = 6*128 = 768 >= 4*20*9
NPOS = NGRP * 128
NMETA = 16                  # f32 slots per positive cell
STRIDE = np.float32(8.0)
IMG = np.float32(640.0)
EPS = np.float32(1e-7)
PI2 = np.float32(np.pi ** 2)
ANCHORS = np.array([[10.0, 13.0], [16.0, 30.0], [33.0, 23.0]], dtype=np.float32)

# atan(z)/z ~ poly(z^2) on [0,1], max err ~6e-7 (f32 horner)
ATAN_C = [0.9999993278352405, -0.33326374521881663, 0.1987987215570962,
          -0.1348040560754345, 0.08374155654506504, -0.03689862924626238,
          0.007825482945513086]

# streaming config (full mode): NT tiles of [128 partitions x KC cells]
NT = 12
KC = NCELL // (NT * 128)    # 50 cells/partition/tile
NTS = 4                     # strided mode: 4 tiles of [128 x 150]
KS = NCELL // (NTS * 128)

# partial-sum column map (out[0, k])
COL_CORR, COL_BOX, COL_CLS, NCOL = 12, 13, 14, 16

MODE = "full"               # "full" (stream all of p_raw) or "strided" (ch4 only)
TRACE = False
TRACE_KW = {}
LAST_RESULT = None

_BUILD_CACHE = {}


def _split_multi_waits(nc, limit=1):
    """This container's walrus build accepts only one sync-wait per
    instruction; split Tile's stacked waits into single-wait NoOp chains."""
    n = 0
    for fn in nc.m.functions:
        for bb in fn.blocks:
            new_insts, changed = [], False
            for inst in bb.instructions:
                si = getattr(inst, "sync_info", None)
                waits = list(si.on_wait) if si is not None and si.on_wait else []
                if len(waits) > limit:
                    changed = True
                    n += 1
                    for w in waits[:-limit]:
                        nop = mybir.InstNoOp(
                            name=nc.get_next_instruction_name(),
                            engine=inst.engine,
                            sync_info=mybir.SyncInfo(on_wait=[w], on_update=[]),
                            bass_nofuse=True,
                        )
                        nc.register_instruction(nop)
                        new_insts.append(nop)
                    si.on_wait = waits[-limit:]
                new_insts.append(inst)
            if changed:
                try:
                    bb.instructions = new_insts
                except Exception:
                    bb.instructions[:] = new_insts
    return n


def _build(mode):
    nc = bass.Bass()
    p = nc.declare_dram_parameter("p", [NCELL, CH], f32, isOutput=False)
    idx = nc.declare_dram_parameter("idx", [128, NGRP], i32, isOutput=False)
    meta = nc.declare_dram_parameter("meta", [128, NGRP * NMETA], f32, isOutput=False)
    tcls = nc.declare_dram_parameter("tcls", [128, NGRP * C], f32, isOutput=False)
    wq = nc.declare_dram_parameter("wq", [128, NGRP * C], f32, isOutput=False)
    outp = nc.declare_dram_parameter("out", [1, NCOL], f32, isOutput=True)

    with tile.TileContext(nc) as tc:
        with tc.tile_pool(name="stream", bufs=3) as streamp, \
             tc.tile_pool(name="work", bufs=2) as workp, \
             tc.tile_pool(name="small", bufs=1) as smallp, \
             tc.tile_pool(name="psum", bufs=1, space="PSUM") as psump:

            partials = smallp.tile([128, NCOL], f32)
            nc.vector.memset(partials[:], 0.0)

            # ---------------- dense objectness pass ----------------
            # focal_bce(x, 0) = 0.25 * exp(-1.5*softplus(-x)) * softplus(x)
            def obj_dense(x_ap, n, col):
                e = workp.tile([128, n], f32, tag="e", name="e")
                l = workp.tile([128, n], f32, tag="l", name="l")
                spn = workp.tile([128, n], f32, tag="spn", name="spn")
                g = workp.tile([128, n], f32, tag="g", name="g")
                sc = workp.tile([128, n], f32, tag="sc", name="sc")
                nc.scalar.activation(e[:], x_ap, AF.Exp)             # e^x
                nc.scalar.activation(l[:], e[:], AF.Ln, bias=1.0)    # softplus(x)
                nc.vector.tensor_scalar_min(l[:], l[:], 88.0)
                nc.vector.tensor_sub(spn[:], l[:], x_ap)             # softplus(-x)
                nc.scalar.activation(g[:], spn[:], AF.Exp, scale=-1.5)
                nc.vector.tensor_mul(sc[:], g[:], l[:])
                nc.vector.tensor_reduce(
                    out=partials[:, col:col + 1], in_=sc[:],
                    axis=AX.X, op=ALU.add,
                )

            if mode == "full":
                pt = p[:].rearrange("(t p k) c -> t p (k c)", t=NT, p=128)
                for t in range(NT):
                    xt = streamp.tile([128, KC * CH], f32, tag="xt", name="xt")
                    nc.sync.dma_start(out=xt[:], in_=pt[t])
                    ch4 = xt[:].rearrange("p (k c) -> p k c", c=CH)[:, :, 4]
                    obj_dense(ch4, KC, t)
            else:
                ps4 = p[:].rearrange("(t p k) c -> t p k c", t=NTS, p=128)
                for t in range(NTS):
                    xt = streamp.tile([128, KS], f32, tag="xs", name="xs")
                    nc.sync.dma_start(out=xt[:], in_=ps4[t, :, :, 4])
                    obj_dense(xt[:], KS, t)

            # ---------------- positive-cell pass ----------------
            idx_t = smallp.tile([128, NGRP], i32)
            nc.sync.dma_start(out=idx_t[:], in_=idx[:])
            meta_t = smallp.tile([128, NGRP * NMETA], f32)
            nc.sync.dma_start(out=meta_t[:], in_=meta[:])
            tcls_t = smallp.tile([128, NGRP * C], f32)
            nc.sync.dma_start(out=tcls_t[:], in_=tcls[:])
            wq_t = smallp.tile([128, NGRP * C], f32)
            nc.sync.dma_start(out=wq_t[:], in_=wq[:])

            pos = smallp.tile([128, NGRP * CH], f32)
            pos3 = pos[:].rearrange("p (g c) -> p g c", c=CH)
            for g_ in range(NGRP):
                nc.gpsimd.indirect_dma_start(
                    out=pos3[:, g_, :],
                    out_offset=None,
                    in_=p[:],
                    in_offset=IndirectOffsetOnAxis(ap=idx_t[:, g_:g_ + 1], axis=0),
                )

            m3 = meta_t[:].rearrange("p (g k) -> p g k", k=NMETA)

            def mk(k):
                return m3[:, :, k]

            valid, cx8, cy8, awpx, ahpx = mk(0), mk(1), mk(2), mk(3), mk(4)
            gxm, gym = mk(5), mk(6)
            gx1, gx2, gy1, gy2 = mk(7), mk(8), mk(9), mk(10)
            areag, atg = mk(11), mk(12)

            G = [128, NGRP]

            def t6(tag):
                return workp.tile(G, f32, tag=tag, name=tag)

            # --- objectness correction at positive cells: t goes 0 -> 1 ---
            xo = pos3[:, :, 4]
            eo, lo, spn6 = t6("eo"), t6("lo"), t6("spn6")
            g0, g1, sc6 = t6("g0"), t6("g1"), t6("sc6")
            nc.scalar.activation(eo[:], xo, AF.Exp)
            nc.scalar.activation(lo[:], eo[:], AF.Ln, bias=1.0)
            nc.vector.tensor_scalar_min(lo[:], lo[:], 88.0)          # softplus(x)
            nc.vector.tensor_sub(spn6[:], lo[:], xo)                 # softplus(-x)
            nc.scalar.activation(g0[:], spn6[:], AF.Exp, scale=-1.5)  # s^1.5
            nc.scalar.activation(g1[:], lo[:], AF.Exp, scale=-1.5)   # (1-s)^1.5
            nc.vector.tensor_mul(g0[:], g0[:], lo[:])                # f0/alpha
            nc.vector.tensor_mul(g1[:], g1[:], spn6[:])              # f1/alpha
            nc.vector.tensor_sub(g1[:], g1[:], g0[:])
            nc.vector.tensor_mul(sc6[:], g1[:], valid)
            nc.vector.tensor_reduce(
                out=partials[:, COL_CORR:COL_CORR + 1], in_=sc6[:],
                axis=AX.X, op=ALU.add,
            )

            # --- CIoU box loss at positive cells ---
            sx, sy, pw, ph = t6("sx"), t6("sy"), t6("pw"), t6("ph")
            nc.scalar.activation(sx[:], pos3[:, :, 0], AF.Exp, scale=-1.0)
            nc.vector.tensor_scalar_add(sx[:], sx[:], 1.0)
            nc.vector.reciprocal(sx[:], sx[:])                       # sigmoid(x0)
            nc.scalar.activation(sy[:], pos3[:, :, 1], AF.Exp, scale=-1.0)
            nc.vector.tensor_scalar_add(sy[:], sy[:], 1.0)
            nc.vector.reciprocal(sy[:], sy[:])                       # sigmoid(x1)
            nc.scalar.activation(pw[:], pos3[:, :, 2], AF.Exp)
            nc.scalar.activation(ph[:], pos3[:, :, 3], AF.Exp)
            px, py = t6("px"), t6("py")
            nc.vector.scalar_tensor_tensor(
                out=px[:], in0=sx[:], scalar=8.0, in1=cx8, op0=ALU.mult, op1=ALU.add)
            nc.vector.scalar_tensor_tensor(
                out=py[:], in0=sy[:], scalar=8.0, in1=cy8, op0=ALU.mult, op1=ALU.add)
            nc.vector.tensor_mul(pw[:], pw[:], awpx)
            nc.vector.tensor_mul(ph[:], ph[:], ahpx)
            px1, px2, py1, py2 = t6("px1"), t6("px2"), t6("py1"), t6("py2")
            hw, hh = t6("hw"), t6("hh")
            nc.vector.tensor_scalar_mul(hw[:], pw[:], 0.5)
            nc.vector.tensor_scalar_mul(hh[:], ph[:], 0.5)
            nc.vector.tensor_sub(px1[:], px[:], hw[:])
            nc.vector.tensor_add(px2[:], px[:], hw[:])
            nc.vector.tensor_sub(py1[:], py[:], hh[:])
            nc.vector.tensor_add(py2[:], py[:], hh[:])
            a6, b6, iw, ih = t6("a6"), t6("b6"), t6("iw"), t6("ih")
            nc.vector.tensor_tensor(out=a6[:], in0=px2[:], in1=gx2, op=ALU.min)
            nc.vector.tensor_tensor(out=b6[:], in0=px1[:], in1=gx1, op=ALU.max)
            nc.vector.tensor_sub(iw[:], a6[:], b6[:])
            nc.vector.tensor_scalar_max(iw[:], iw[:], 0.0)
            nc.vector.tensor_tensor(out=a6[:], in0=py2[:], in1=gy2, op=ALU.min)
            nc.vector.tensor_tensor(out=b6[:], in0=py1[:], in1=gy1, op=ALU.max)
            nc.vector.tensor_sub(ih[:], a6[:], b6[:])
            nc.vector.tensor_scalar_max(ih[:], ih[:], 0.0)
            inter = t6("inter")
            nc.vector.tensor_mul(inter[:], iw[:], ih[:])
            # union = relu(px2-px1)*relu(py2-py1) + areag - inter + EPS
            ap_, bp_ = t6("ap_"), t6("bp_")
            nc.vector.tensor_sub(ap_[:], px2[:], px1[:])
            nc.vector.tensor_scalar_max(ap_[:], ap_[:], 0.0)
            nc.vector.tensor_sub(bp_[:], py2[:], py1[:])
            nc.vector.tensor_scalar_max(bp_[:], bp_[:], 0.0)
            union = t6("union")
            nc.vector.tensor_mul(union[:], ap_[:], bp_[:])
            nc.vector.tensor_add(union[:], union[:], areag)
            nc.vector.tensor_sub(union[:], union[:], inter[:])
            nc.vector.tensor_scalar_add(union[:], union[:], float(EPS))
            iou = t6("iou")
            nc.vector.reciprocal(iou[:], union[:])
            nc.vector.tensor_mul(iou[:], inter[:], iou[:])
            # enclosing box diag^2
            cw, chv = t6("cw"), t6("chv")
            nc.vector.tensor_tensor(out=a6[:], in0=px2[:], in1=gx2, op=ALU.max)
            nc.vector.tensor_tensor(out=b6[:], in0=px1[:], in1=gx1, op=ALU.min)
            nc.vector.tensor_sub(cw[:], a6[:], b6[:])
            nc.vector.tensor_scalar_max(cw[:], cw[:], 0.0)
            nc.vector.tensor_tensor(out=a6[:], in0=py2[:], in1=gy2, op=ALU.max)
            nc.vector.tensor_tensor(out=b6[:], in0=py1[:], in1=gy1, op=ALU.min)
            nc.vector.tensor_sub(chv[:], a6[:], b6[:])
            nc.vector.tensor_scalar_max(chv[:], chv[:], 0.0)
            c2 = t6("c2")
            nc.vector.tensor_mul(cw[:], cw[:], cw[:])
            nc.vector.tensor_mul(chv[:], chv[:], chv[:])
            nc.vector.tensor_add(c2[:], cw[:], chv[:])
            nc.vector.tensor_scalar_add(c2[:], c2[:], float(EPS))
            rho2 = t6("rho2")
            nc.vector.tensor_tensor(out=a6[:], in0=px[:], in1=gxm, op=ALU.subtract)
            nc.vector.tensor_mul(a6[:], a6[:], a6[:])
            nc.vector.tensor_tensor(out=b6[:], in0=py[:], in1=gym, op=ALU.subtract)
            nc.vector.tensor_mul(b6[:], b6[:], b6[:])
            nc.vector.tensor_add(rho2[:], a6[:], b6[:])
            # atan(pw/(ph+EPS)) via polynomial (no trig table)
            q, qi, z, z2 = t6("q"), t6("qi"), t6("z"), t6("z2")
            nc.vector.tensor_scalar_add(q[:], ph[:], float(EPS))
            nc.vector.reciprocal(q[:], q[:])
            nc.vector.tensor_mul(q[:], pw[:], q[:])                  # q > 0
            nc.vector.reciprocal(qi[:], q[:])
            nc.vector.tensor_tensor(out=z[:], in0=q[:], in1=qi[:], op=ALU.min)
            nc.vector.tensor_mul(z2[:], z[:], z[:])
            acc = t6("acc")
            nc.vector.tensor_scalar(
                out=acc[:], in0=z2[:], scalar1=float(ATAN_C[6]),
                scalar2=float(ATAN_C[5]), op0=ALU.mult, op1=ALU.add)
            for k in (4, 3, 2, 1, 0):
                nc.vector.tensor_mul(acc[:], acc[:], z2[:])
                nc.vector.tensor_scalar_add(acc[:], acc[:], float(ATAN_C[k]))
            nc.vector.tensor_mul(acc[:], acc[:], z[:])               # atan(z)
            flag = t6("flag")
            nc.vector.tensor_scalar(
                out=flag[:], in0=q[:], scalar1=1.0, scalar2=None, op0=ALU.is_gt)
            fw = t6("fw")
            nc.vector.tensor_scalar(
                out=fw[:], in0=acc[:], scalar1=-2.0,
                scalar2=float(np.pi / 2), op0=ALU.mult, op1=ALU.add)
            nc.vector.tensor_mul(fw[:], fw[:], flag[:])
            nc.vector.tensor_add(acc[:], acc[:], fw[:])              # atan(q)
            vv = t6("vv")
            nc.vector.tensor_tensor(out=vv[:], in0=atg, in1=acc[:], op=ALU.subtract)
            nc.vector.tensor_mul(vv[:], vv[:], vv[:])
            nc.vector.tensor_scalar_mul(vv[:], vv[:], float(np.float32(4.0) / PI2))
            # alpha = v / (1 - iou + v + EPS)
            den = t6("den")
            nc.vector.scalar_tensor_tensor(
                out=den[:], in0=iou[:], scalar=-1.0, in1=vv[:],
                op0=ALU.mult, op1=ALU.add)
            nc.vector.tensor_scalar_add(den[:], den[:], float(1.0 + float(EPS)))
            nc.vector.reciprocal(den[:], den[:])
            nc.vector.tensor_mul(den[:], vv[:], den[:])              # alpha
            nc.vector.tensor_mul(den[:], den[:], vv[:])              # alpha*v
            # loss = 1 - iou + rho2/c2 + alpha*v
            nc.vector.reciprocal(c2[:], c2[:])
            nc.vector.tensor_mul(rho2[:], rho2[:], c2[:])
            nc.vector.tensor_add(den[:], den[:], rho2[:])
            nc.vector.tensor_sub(den[:], den[:], iou[:])
            nc.vector.tensor_scalar_add(den[:], den[:], 1.0)
            bsc = t6("bsc")
            nc.vector.tensor_mul(bsc[:], den[:], valid)
            nc.vector.tensor_reduce(
                out=partials[:, COL_BOX:COL_BOX + 1], in_=bsc[:],
                axis=AX.X, op=ALU.add,
            )

            # --- weighted focal class loss at positive cells ---
            NCL = NGRP * C
            xc = pos3[:, :, 5:]                                      # [128,6,80]

            def tcl(name):
                return smallp.tile([128, NCL], f32, name=name)

            ecl, scl, lcl = tcl("ecl"), tcl("scl"), tcl("lcl")
            ucl, fcl, sccl = tcl("ucl"), tcl("fcl"), tcl("sccl")
            e3 = ecl[:].rearrange("p (g c) -> p g c", c=C)
            nc.scalar.activation(e3, xc, AF.Exp)                     # e^x
            nc.vector.tensor_scalar_add(scl[:], ecl[:], 1.0)
            nc.vector.reciprocal(scl[:], scl[:])                     # 1 - sigmoid
            nc.vector.tensor_scalar(
                out=scl[:], in0=scl[:], scalar1=-1.0, scalar2=1.0,
                op0=ALU.mult, op1=ALU.add)                           # sigmoid
            nc.scalar.activation(lcl[:], ecl[:], AF.Ln, bias=1.0)    # softplus
            nc.vector.tensor_scalar_min(lcl[:], lcl[:], 88.0)
            nc.vector.tensor_mul(ucl[:], scl[:], tcls_t[:])          # s*t
            nc.vector.scalar_tensor_tensor(
                out=ucl[:], in0=ucl[:], scalar=-2.0, in1=scl[:],
                op0=ALU.mult, op1=ALU.add)                           # s - 2st
            nc.vector.tensor_add(ucl[:], ucl[:], tcls_t[:])          # u
            nc.vector.tensor_scalar_max(ucl[:], ucl[:], 1e-38)
            nc.scalar.activation(ucl[:], ucl[:], AF.Ln)
            nc.scalar.activation(ucl[:], ucl[:], AF.Exp, scale=1.5)  # u^1.5
            f3 = fcl[:].rearrange("p (g c) -> p g c", c=C)
            nc.vector.tensor_tensor(out=f3, in0=xc, in1=tcls_t[:].rearrange(
                "p (g c) -> p g c", c=C), op=ALU.mult)               # x*t
            nc.vector.tensor_sub(fcl[:], lcl[:], fcl[:])             # bce
            nc.vector.tensor_mul(fcl[:], ucl[:], fcl[:])
            nc.vector.tensor_mul(sccl[:], fcl[:], wq_t[:])
            nc.vector.tensor_reduce(
                out=partials[:, COL_CLS:COL_CLS + 1], in_=sccl[:],
                axis=AX.X, op=ALU.add,
            )

            # ---------------- cross-partition reduce + store ----------------
            ones = smallp.tile([128, 1], f32)
            nc.vector.memset(ones[:], 1.0)
            ps = psump.tile([1, NCOL], f32)
            nc.tensor.matmul(out=ps[:], lhsT=ones[:], rhs=partials[:],
                             start=True, stop=True)
            res = smallp.tile([1, NCOL], f32)
            nc.vector.tensor_copy(out=res[:], in_=ps[:])
            nc.sync.dma_start(out=outp[:], in_=res[:])

    _split_multi_waits(nc)
    return nc


def _assign_targets_host(labels, label_mask, cls_weight):
    """Replicate reference.assign_targets scatter on host; returns per-core
    device aux inputs and global n_pos."""
    labels = np.asarray(labels, dtype=np.float32)
    mask = np.asarray(label_mask).astype(bool)
    cw = np.asarray(cls_weight, dtype=np.float32)

    gcls = labels[..., 0].astype(np.int32)                      # [B, M]
    gx = labels[..., 1] * IMG
    gy = labels[..., 2] * IMG
    gw = labels[..., 3] * IMG
    gh = labels[..., 4] * IMG
    gi = np.clip(gx / STRIDE, np.float32(0.0), np.float32(W - 0.001)).astype(np.int32)
    gj = np.clip(gy / STRIDE, np.float32(0.0), np.float32(H - 0.001)).astype(np.int32)
    gtw, gth = gw / STRIDE, gh / STRIDE
    ag = ANCHORS / STRIDE                                       # [3, 2]
    inter = np.minimum(gtw[..., None], ag[:, 0]) * np.minimum(gth[..., None], ag[:, 1])
    union = gtw[..., None] * gth[..., None] + ag[:, 0] * ag[:, 1] - inter + np.float32(1e-9)
    best_a = np.argmax(inter / union, axis=-1).astype(np.int32)  # [B, M]

    offs = [(di, dj) for di in (-1, 0, 1) for dj in (-1, 0, 1)]
    # sequential scatter with last-write-wins box, accumulating class set
    targets = {}  # (b, a, j, i) -> [set(cls), (bx, by, bw, bh)]
    for b in range(B):
        for m in range(M):
            if not mask[b, m]:
                continue
            a = int(best_a[b, m])
            c = int(gcls[b, m])
            box = (gx[b, m], gy[b, m], gw[b, m], gh[b, m])
            for di, dj in offs:
                i = min(max(int(gi[b, m]) + di, 0), W - 1)
                j = min(max(int(gj[b, m]) + dj, 0), H - 1)
                e = targets.setdefault((b, a, j, i), [set(), None])
                e[0].add(c)
                e[1] = box
    n_pos = max(len(targets), 1)

    idx_all = np.zeros((NCORES, 128, NGRP), dtype=np.int32)
    meta_all = np.zeros((NCORES, 128, NGRP * NMETA), dtype=np.float32)
    tcls_all = np.zeros((NCORES, 128, NGRP * C), dtype=np.float32)
    wq_all = np.zeros((NCORES, 128, NGRP * C), dtype=np.float32)
    slot_ctr = [0] * NCORES
    for (b, a, j, i), (clsset, box) in targets.items():
        core = b // BL
        s = slot_ctr[core]
        slot_ctr[core] += 1
        assert s < NPOS, "positive-cell capacity exceeded"
        p_, g_ = s % 128, s // 128
        bloc = b - core * BL
        idx_all[core, p_, g_] = ((bloc * NA + a) * H + j) * W + i
        bx, by, bw, bh = box
        gx1 = bx - bw * np.float32(0.5)
        gx2 = bx + bw * np.float32(0.5)
        gy1 = by - bh * np.float32(0.5)
        gy2 = by + bh * np.float32(0.5)
        areag = max(gx2 - gx1, np.float32(0.0)) * max(gy2 - gy1, np.float32(0.0))
        atg = np.float32(np.arctan(bw / (bh + EPS)))
        mslot = np.array(
            [1.0, i * 8.0, j * 8.0, ANCHORS[a, 0], ANCHORS[a, 1],
             bx, by, gx1, gx2, gy1, gy2, areag, atg, 0.0, 0.0, 0.0],
            dtype=np.float32,
        )
        meta_all[core, p_, g_ * NMETA:(g_ + 1) * NMETA] = mslot
        for c in clsset:
            tcls_all[core, p_, g_ * C + c] = 1.0
        wq_all[core, p_, g_ * C:(g_ + 1) * C] = np.float32(0.25) * cw
    return idx_all, meta_all, tcls_all, wq_all, n_pos


def kernel(p_raw, labels, label_mask, cls_weight):
    global LAST_RESULT
    p_raw = np.ascontiguousarray(np.asarray(p_raw, dtype=np.float32))
    idx_all, meta_all, tcls_all, wq_all, n_pos = _assign_targets_host(
        labels, label_mask, cls_weight
    )

    if MODE not in _BUILD_CACHE:
        _BUILD_CACHE[MODE] = _build(MODE)
    nc = _BUILD_CACHE[MODE]

    shards = p_raw.reshape(NCORES, NCELL, CH)
    in_maps = []
    for c in range(NCORES):
        in_maps.append({
            "p": shards[c],
            "idx": idx_all[c],
            "meta": meta_all[c],
            "tcls": tcls_all[c],
            "wq": wq_all[c],
        })

    r = run_bass_kernel_spmd(
        nc, in_maps, core_ids=list(range(NCORES)), trace=TRACE, **TRACE_KW
    )
    LAST_RESULT = r

    outs = np.stack([np.asarray(r.results[c]["out"][0]) for c in range(NCORES)])
    sums = outs.astype(np.float64).sum(axis=0)
    s_dense = sums[:COL_CORR].sum()
    l_obj = 0.25 * (s_dense + sums[COL_CORR]) / float(B * NA * H * W)
    l_box = sums[COL_BOX] / n_pos
    l_cls = sums[COL_CLS] / (n_pos * C)
    total = 7.5 * l_box + 1.0 * l_obj + 0.5 * l_cls
    return np.float32(total)


# revision 6
# speedup vs baseline: 1.0390x; 1.0390x over previous
"""Trainium2 Bass kernel for nn_DBLoss (YOLO-style detection loss).

Strategy (pure data parallel over batch, 8 cores x 4 images):
  * The loss decomposes as 7.5*l_box + l_obj + 0.5*l_cls where only the
    objectness term touches every grid cell; box/cls terms only touch the
    ~180 label-assigned cells per image.
  * Host (numpy) replicates the reference's target assignment on the tiny
    `labels` tensor to produce per-core scatter metadata: positive-cell
    indices, gt-box constants, multi-hot class targets.  Collision
    semantics match the reference scatter: tbox last-write-wins, tcls
    accumulates classes (class is part of the scatter index).
  * Device: streams the p_raw shard to compute sum(focal_bce(obj_logit, 0))
    over all cells, gathers positive cells by indirect DMA, computes the
    obj t=1 correction, CIoU box loss and weighted focal cls loss there,
    and emits per-core partial sums.
  * Host sums 8x16 partials and applies the n_pos / mean normalizations.

All transcendentals use only the Exp and Ln ACT LUTs (one table set:
natural_log_exp_and_others), so a single act-table load suffices:
  softplus(x)        = ln(1 + exp(x))            (clamped at 88)
  sigmoid(x)^1.5     = exp(-1.5 * softplus(-x))
  (1-sigmoid(x))^1.5 = exp(-1.5 * softplus(x))
  sigmoid(x)         = 1/(1 + exp(-x))           (DVE reciprocal is exact)
  u^1.5              = exp(1.5 * ln(max(u, tiny)))
  arctan             = odd polynomial in z^2 after range reduction (DVE)
"""

import sys

sys.path.insert(0, "/opt/trn_rl_repo")

import numpy as np

import concourse.bass as bass
import concourse.tile as tile
from concourse import mybir
from concourse.bass import IndirectOffsetOnAxis
from concourse.bass_utils import run_bass_kernel_spmd

f32 = mybir.dt.float32
i32 = mybir.dt.int32
AF = mybir.ActivationFunctionType
ALU = mybir.AluOpType
AX = mybir.AxisListType

# problem constants (hardcoded per harness contract)
B, NA, H, W, M, C = 32, 3, 80, 80, 20, 80
CH = 5 + C
NCORES = 8
BL = B // NCORES            # 4 images per core
NCELL = BL * NA * H * W     # 76800 cells per core
NGRP = 6                    # positive-cell capacity = 6*128 = 768 >= 4*20*9
NPOS = NGRP * 128
NMETA = 16                  # f32 slots per positive cell
STRIDE = np.float32(8.0)
IMG = np.float32(640.0)
EPS = np.float32(1e-7)
PI2 = np.float32(np.pi ** 2)
ANCHORS = np.array([[10.0, 13.0], [16.0, 30.0], [33.0, 23.0]], dtype=np.float32)

# atan(z)/z ~ poly(z^2) on [0,1], max err ~6e-7 (f32 horner)
ATAN_C = [0.9999993278352405, -0.33326374521881663, 0.1987987215570962,
          -0.1348040560754345, 0.08374155654506504, -0.03689862924626238,
          0.007825482945513086]

# streaming config (full mode): NT tiles of [128 partitions x KC cells]
NT = 12
KC = NCELL // (NT * 128)    # 50 cells/partition/tile
NTS = 4                     # strided mode: 4 tiles of [128 x 150]
KS = NCELL // (NTS * 128)

# partial-sum column map (out[0, k])
COL_CORR, COL_BOX, COL_CLS, NCOL = 12, 13, 14, 16

MODE = "full"               # "full" (stream all of p_raw) or "strided" (ch4 only)
TRACE = False
TRACE_KW = {}
LAST_RESULT = None

_BUILD_CACHE = {}


def _split_multi_waits(nc, limit=1):
    """This container's walrus build accepts only one sync-wait per
    instruction; split Tile's stacked waits into single-wait NoOp chains."""
    n = 0
    for fn in nc.m.functions:
        for bb in fn.blocks:
            new_insts, changed = [], False
            for inst in bb.instructions:
                si = getattr(inst, "sync_info", None)
                waits = list(si.on_wait) if si is not None and si.on_wait else []
                if len(waits) > limit:
                    changed = True
                    n += 1
                    for w in waits[:-limit]:
                        nop = mybir.InstNoOp(
                            name=nc.get_next_instruction_name(),
                            engine=inst.engine,
                            sync_info=mybir.SyncInfo(on_wait=[w], on_update=[]),
                            bass_nofuse=True,
                        )
                        nc.register_instruction(nop)
                        new_insts.append(nop)
                    si.on_wait = waits[-limit:]
                new_insts.append(inst)
            if changed:
                try:
                    bb.instructions = new_insts
                except Exception:
                    bb.instructions[:] = new_insts
    return n


def _build(mode):
    nc = bass.Bass()
    p = nc.declare_dram_parameter("p", [NCELL, CH], f32, isOutput=False)
    idx = nc.declare_dram_parameter("idx", [128, NGRP], i32, isOutput=False)
    meta = nc.declare_dram_parameter("meta", [128, NGRP * NMETA], f32, isOutput=False)
    tcls = nc.declare_dram_parameter("tcls", [128, NGRP * C], f32, isOutput=False)
    wq = nc.declare_dram_parameter("wq", [128, NGRP * C], f32, isOutput=False)
    outp = nc.declare_dram_parameter("out", [1, NCOL], f32, isOutput=True)

    with tile.TileContext(nc) as tc:
        with tc.tile_pool(name="stream", bufs=3) as streamp, \
             tc.tile_pool(name="work", bufs=2) as workp, \
             tc.tile_pool(name="small", bufs=1) as smallp, \
             tc.tile_pool(name="psum", bufs=1, space="PSUM") as psump:

            partials = smallp.tile([128, NCOL], f32)
            nc.vector.memset(partials[:], 0.0)

            # ---------------- dense objectness pass ----------------
            # focal_bce(x, 0) = 0.25 * exp(-1.5*softplus(-x)) * softplus(x)
            def obj_dense(x_ap, n, col):
                e = workp.tile([128, n], f32, tag="e", name="e")
                l = workp.tile([128, n], f32, tag="l", name="l")
                spn = workp.tile([128, n], f32, tag="spn", name="spn")
                g = workp.tile([128, n], f32, tag="g", name="g")
                sc = workp.tile([128, n], f32, tag="sc", name="sc")
                nc.scalar.activation(e[:], x_ap, AF.Exp)             # e^x
                nc.scalar.activation(l[:], e[:], AF.Ln, bias=1.0)    # softplus(x)
                nc.vector.tensor_scalar_min(l[:], l[:], 88.0)
                nc.vector.tensor_sub(spn[:], l[:], x_ap)             # softplus(-x)
                nc.scalar.activation(g[:], spn[:], AF.Exp, scale=-1.5)
                nc.vector.tensor_mul(sc[:], g[:], l[:])
                nc.vector.tensor_reduce(
                    out=partials[:, col:col + 1], in_=sc[:],
                    axis=AX.X, op=ALU.add,
                )

            if mode == "full":
                pt = p[:].rearrange("(t p k) c -> t p (k c)", t=NT, p=128)
                for t in range(NT):
                    xt = streamp.tile([128, KC * CH], f32, tag="xt", name="xt")
                    nc.sync.dma_start(out=xt[:], in_=pt[t])
                    ch4 = xt[:].rearrange("p (k c) -> p k c", c=CH)[:, :, 4]
                    obj_dense(ch4, KC, t)
            elif mode == "strided":
                ps4 = p[:].rearrange("(t p k) c -> t p k c", t=NTS, p=128)
                for t in range(NTS):
                    xt = streamp.tile([128, KS], f32, tag="xs", name="xs")
                    nc.sync.dma_start(out=xt[:], in_=ps4[t, :, :, 4])
                    obj_dense(xt[:], KS, t)
            else:  # strided2: split ch4 extraction over both HWDGE rings
                NT2 = 8
                K2 = NCELL // (NT2 * 128)
                ps8 = p[:].rearrange("(t p k) c -> t p k c", t=NT2, p=128)
                for t in range(NT2):
                    xt = streamp.tile([128, K2], f32, tag="xs2", name="xs2")
                    eng = nc.sync if t % 2 == 0 else nc.scalar
                    eng.dma_start(out=xt[:], in_=ps8[t, :, :, 4])
                    obj_dense(xt[:], K2, t)

            # ---------------- positive-cell pass ----------------
            idx_t = smallp.tile([128, NGRP], i32)
            nc.sync.dma_start(out=idx_t[:], in_=idx[:])
            meta_t = smallp.tile([128, NGRP * NMETA], f32)
            nc.sync.dma_start(out=meta_t[:], in_=meta[:])
            tcls_t = smallp.tile([128, NGRP * C], f32)
            nc.sync.dma_start(out=tcls_t[:], in_=tcls[:])
            wq_t = smallp.tile([128, NGRP * C], f32)
            nc.sync.dma_start(out=wq_t[:], in_=wq[:])

            pos = smallp.tile([128, NGRP * CH], f32)
            pos3 = pos[:].rearrange("p (g c) -> p g c", c=CH)
            for g_ in range(NGRP):
                nc.gpsimd.indirect_dma_start(
                    out=pos3[:, g_, :],
                    out_offset=None,
                    in_=p[:],
                    in_offset=IndirectOffsetOnAxis(ap=idx_t[:, g_:g_ + 1], axis=0),
                )

            m3 = meta_t[:].rearrange("p (g k) -> p g k", k=NMETA)

            def mk(k):
                return m3[:, :, k]

            valid, cx8, cy8, awpx, ahpx = mk(0), mk(1), mk(2), mk(3), mk(4)
            gxm, gym = mk(5), mk(6)
            gx1, gx2, gy1, gy2 = mk(7), mk(8), mk(9), mk(10)
            areag, atg = mk(11), mk(12)

            G = [128, NGRP]

            def t6(tag):
                return workp.tile(G, f32, tag=tag, name=tag)

            # --- objectness correction at positive cells: t goes 0 -> 1 ---
            xo = pos3[:, :, 4]
            eo, lo, spn6 = t6("eo"), t6("lo"), t6("spn6")
            g0, g1, sc6 = t6("g0"), t6("g1"), t6("sc6")
            nc.scalar.activation(eo[:], xo, AF.Exp)
            nc.scalar.activation(lo[:], eo[:], AF.Ln, bias=1.0)
            nc.vector.tensor_scalar_min(lo[:], lo[:], 88.0)          # softplus(x)
            nc.vector.tensor_sub(spn6[:], lo[:], xo)                 # softplus(-x)
            nc.scalar.activation(g0[:], spn6[:], AF.Exp, scale=-1.5)  # s^1.5
            nc.scalar.activation(g1[:], lo[:], AF.Exp, scale=-1.5)   # (1-s)^1.5
            nc.vector.tensor_mul(g0[:], g0[:], lo[:])                # f0/alpha
            nc.vector.tensor_mul(g1[:], g1[:], spn6[:])              # f1/alpha
            nc.vector.tensor_sub(g1[:], g1[:], g0[:])
            nc.vector.tensor_mul(sc6[:], g1[:], valid)
            nc.vector.tensor_reduce(
                out=partials[:, COL_CORR:COL_CORR + 1], in_=sc6[:],
                axis=AX.X, op=ALU.add,
            )

            # --- CIoU box loss at positive cells ---
            sx, sy, pw, ph = t6("sx"), t6("sy"), t6("pw"), t6("ph")
            nc.scalar.activation(sx[:], pos3[:, :, 0], AF.Exp, scale=-1.0)
            nc.vector.tensor_scalar_add(sx[:], sx[:], 1.0)
            nc.vector.reciprocal(sx[:], sx[:])                       # sigmoid(x0)
            nc.scalar.activation(sy[:], pos3[:, :, 1], AF.Exp, scale=-1.0)
            nc.vector.tensor_scalar_add(sy[:], sy[:], 1.0)
            nc.vector.reciprocal(sy[:], sy[:])                       # sigmoid(x1)
            nc.scalar.activation(pw[:], pos3[:, :, 2], AF.Exp)
            nc.scalar.activation(ph[:], pos3[:, :, 3], AF.Exp)
            px, py = t6("px"), t6("py")
            nc.vector.scalar_tensor_tensor(
                out=px[:], in0=sx[:], scalar=8.0, in1=cx8, op0=ALU.mult, op1=ALU.add)
            nc.vector.scalar_tensor_tensor(
                out=py[:], in0=sy[:], scalar=8.0, in1=cy8, op0=ALU.mult, op1=ALU.add)
            nc.vector.tensor_mul(pw[:], pw[:], awpx)
            nc.vector.tensor_mul(ph[:], ph[:], ahpx)
            px1, px2, py1, py2 = t6("px1"), t6("px2"), t6("py1"), t6("py2")
            hw, hh = t6("hw"), t6("hh")
            nc.vector.tensor_scalar_mul(hw[:], pw[:], 0.5)
            nc.vector.tensor_scalar_mul(hh[:], ph[:], 0.5)
            nc.vector.tensor_sub(px1[:], px[:], hw[:])
            nc.vector.tensor_add(px2[:], px[:], hw[:])
            nc.vector.tensor_sub(py1[:], py[:], hh[:])
            nc.vector.tensor_add(py2[:], py[:], hh[:])
            a6, b6, iw, ih = t6("a6"), t6("b6"), t6("iw"), t6("ih")
            nc.vector.tensor_tensor(out=a6[:], in0=px2[:], in1=gx2, op=ALU.min)
            nc.vector.tensor_tensor(out=b6[:], in0=px1[:], in1=gx1, op=ALU.max)
            nc.vector.tensor_sub(iw[:], a6[:], b6[:])
            nc.vector.tensor_scalar_max(iw[:], iw[:], 0.0)
            nc.vector.tensor_tensor(out=a6[:], in0=py2[:], in1=gy2, op=ALU.min)
            nc.vector.tensor_tensor(out=b6[:], in0=py1[:], in1=gy1, op=ALU.max)
            nc.vector.tensor_sub(ih[:], a6[:], b6[:])
            nc.vector.tensor_scalar_max(ih[:], ih[:], 0.0)
            inter = t6("inter")
            nc.vector.tensor_mul(inter[:], iw[:], ih[:])
            # union = relu(px2-px1)*relu(py2-py1) + areag - inter + EPS
            ap_, bp_ = t6("ap_"), t6("bp_")
            nc.vector.tensor_sub(ap_[:], px2[:], px1[:])
            nc.vector.tensor_scalar_max(ap_[:], ap_[:], 0.0)
            nc.vector.tensor_sub(bp_[:], py2[:], py1[:])
            nc.vector.tensor_scalar_max(bp_[:], bp_[:], 0.0)
            union = t6("union")
            nc.vector.tensor_mul(union[:], ap_[:], bp_[:])
            nc.vector.tensor_add(union[:], union[:], areag)
            nc.vector.tensor_sub(union[:], union[:], inter[:])
            nc.vector.tensor_scalar_add(union[:], union[:], float(EPS))
            iou = t6("iou")
            nc.vector.reciprocal(iou[:], union[:])
            nc.vector.tensor_mul(iou[:], inter[:], iou[:])
            # enclosing box diag^2
            cw, chv = t6("cw"), t6("chv")
            nc.vector.tensor_tensor(out=a6[:], in0=px2[:], in1=gx2, op=ALU.max)
            nc.vector.tensor_tensor(out=b6[:], in0=px1[:], in1=gx1, op=ALU.min)
            nc.vector.tensor_sub(cw[:], a6[:], b6[:])
            nc.vector.tensor_scalar_max(cw[:], cw[:], 0.0)
            nc.vector.tensor_tensor(out=a6[:], in0=py2[:], in1=gy2, op=ALU.max)
            nc.vector.tensor_tensor(out=b6[:], in0=py1[:], in1=gy1, op=ALU.min)
            nc.vector.tensor_sub(chv[:], a6[:], b6[:])
            nc.vector.tensor_scalar_max(chv[:], chv[:], 0.0)
            c2 = t6("c2")
            nc.vector.tensor_mul(cw[:], cw[:], cw[:])
            nc.vector.tensor_mul(chv[:], chv[:], chv[:])
            nc.vector.tensor_add(c2[:], cw[:], chv[:])
            nc.vector.tensor_scalar_add(c2[:], c2[:], float(EPS))
            rho2 = t6("rho2")
            nc.vector.tensor_tensor(out=a6[:], in0=px[:], in1=gxm, op=ALU.subtract)
            nc.vector.tensor_mul(a6[:], a6[:], a6[:])
            nc.vector.tensor_tensor(out=b6[:], in0=py[:], in1=gym, op=ALU.subtract)
            nc.vector.tensor_mul(b6[:], b6[:], b6[:])
            nc.vector.tensor_add(rho2[:], a6[:], b6[:])
            # atan(pw/(ph+EPS)) via polynomial (no trig table)
            q, qi, z, z2 = t6("q"), t6("qi"), t6("z"), t6("z2")
            nc.vector.tensor_scalar_add(q[:], ph[:], float(EPS))
            nc.vector.reciprocal(q[:], q[:])
            nc.vector.tensor_mul(q[:], pw[:], q[:])                  # q > 0
            nc.vector.reciprocal(qi[:], q[:])
            nc.vector.tensor_tensor(out=z[:], in0=q[:], in1=qi[:], op=ALU.min)
            nc.vector.tensor_mul(z2[:], z[:], z[:])
            acc = t6("acc")
            nc.vector.tensor_scalar(
                out=acc[:], in0=z2[:], scalar1=float(ATAN_C[6]),
                scalar2=float(ATAN_C[5]), op0=ALU.mult, op1=ALU.add)
            for k in (4, 3, 2, 1, 0):
                nc.vector.tensor_mul(acc[:], acc[:], z2[:])
                nc.vector.tensor_scalar_add(acc[:], acc[:], float(ATAN_C[k]))
            nc.vector.tensor_mul(acc[:], acc[:], z[:])               # atan(z)
            flag = t6("flag")
            nc.vector.tensor_scalar(
                out=flag[:], in0=q[:], scalar1=1.0, scalar2=None, op0=ALU.is_gt)
            fw = t6("fw")
            nc.vector.tensor_scalar(
                out=fw[:], in0=acc[:], scalar1=-2.0,
                scalar2=float(np.pi / 2), op0=ALU.mult, op1=ALU.add)
            nc.vector.tensor_mul(fw[:], fw[:], flag[:])
            nc.vector.tensor_add(acc[:], acc[:], fw[:])              # atan(q)
            vv = t6("vv")
            nc.vector.tensor_tensor(out=vv[:], in0=atg, in1=acc[:], op=ALU.subtract)
            nc.vector.tensor_mul(vv[:], vv[:], vv[:])
            nc.vector.tensor_scalar_mul(vv[:], vv[:], float(np.float32(4.0) / PI2))
            # alpha = v / (1 - iou + v + EPS)
            den = t6("den")
            nc.vector.scalar_tensor_tensor(
                out=den[:], in0=iou[:], scalar=-1.0, in1=vv[:],
                op0=ALU.mult, op1=ALU.add)
            nc.vector.tensor_scalar_add(den[:], den[:], float(1.0 + float(EPS)))
            nc.vector.reciprocal(den[:], den[:])
            nc.vector.tensor_mul(den[:], vv[:], den[:])              # alpha
            nc.vector.tensor_mul(den[:], den[:], vv[:])              # alpha*v
            # loss = 1 - iou + rho2/c2 + alpha*v
            nc.vector.reciprocal(c2[:], c2[:])
            nc.vector.tensor_mul(rho2[:], rho2[:], c2[:])
            nc.vector.tensor_add(den[:], den[:], rho2[:])
            nc.vector.tensor_sub(den[:], den[:], iou[:])
            nc.vector.tensor_scalar_add(den[:], den[:], 1.0)
            bsc = t6("bsc")
            nc.vector.tensor_mul(bsc[:], den[:], valid)
            nc.vector.tensor_reduce(
                out=partials[:, COL_BOX:COL_BOX + 1], in_=bsc[:],
                axis=AX.X, op=ALU.add,
            )

            # --- weighted focal class loss at positive cells ---
            NCL = NGRP * C
            xc = pos3[:, :, 5:]                                      # [128,6,80]

            def tcl(name):
                return smallp.tile([128, NCL], f32, name=name)

            ecl, scl, lcl = tcl("ecl"), tcl("scl"), tcl("lcl")
            ucl, fcl, sccl = tcl("ucl"), tcl("fcl"), tcl("sccl")
            e3 = ecl[:].rearrange("p (g c) -> p g c", c=C)
            nc.scalar.activation(e3, xc, AF.Exp)                     # e^x
            nc.vector.tensor_scalar_add(scl[:], ecl[:], 1.0)
            nc.vector.reciprocal(scl[:], scl[:])                     # 1 - sigmoid
            nc.vector.tensor_scalar(
                out=scl[:], in0=scl[:], scalar1=-1.0, scalar2=1.0,
                op0=ALU.mult, op1=ALU.add)                           # sigmoid
            nc.scalar.activation(lcl[:], ecl[:], AF.Ln, bias=1.0)    # softplus
            nc.vector.tensor_scalar_min(lcl[:], lcl[:], 88.0)
            nc.vector.tensor_mul(ucl[:], scl[:], tcls_t[:])          # s*t
            nc.vector.scalar_tensor_tensor(
                out=ucl[:], in0=ucl[:], scalar=-2.0, in1=scl[:],
                op0=ALU.mult, op1=ALU.add)                           # s - 2st
            nc.vector.tensor_add(ucl[:], ucl[:], tcls_t[:])          # u
            nc.vector.tensor_scalar_max(ucl[:], ucl[:], 1e-38)
            nc.scalar.activation(ucl[:], ucl[:], AF.Ln)
            nc.scalar.activation(ucl[:], ucl[:], AF.Exp, scale=1.5)  # u^1.5
            f3 = fcl[:].rearrange("p (g c) -> p g c", c=C)
            nc.vector.tensor_tensor(out=f3, in0=xc, in1=tcls_t[:].rearrange(
                "p (g c) -> p g c", c=C), op=ALU.mult)               # x*t
            nc.vector.tensor_sub(fcl[:], lcl[:], fcl[:])             # bce
            nc.vector.tensor_mul(fcl[:], ucl[:], fcl[:])
            nc.vector.tensor_mul(sccl[:], fcl[:], wq_t[:])
            nc.vector.tensor_reduce(
                out=partials[:, COL_CLS:COL_CLS + 1], in_=sccl[:],
                axis=AX.X, op=ALU.add,
            )

            # ---------------- cross-partition reduce + store ----------------
            ones = smallp.tile([128, 1], f32)
            nc.vector.memset(ones[:], 1.0)
            ps = psump.tile([1, NCOL], f32)
            nc.tensor.matmul(out=ps[:], lhsT=ones[:], rhs=partials[:],
                             start=True, stop=True)
            res = smallp.tile([1, NCOL], f32)
            nc.vector.tensor_copy(out=res[:], in_=ps[:])
            nc.sync.dma_start(out=outp[:], in_=res[:])

    _split_multi_waits(nc)
    return nc


def _assign_targets_host(labels, label_mask, cls_weight):
    """Replicate reference.assign_targets scatter on host; returns per-core
    device aux inputs and global n_pos."""
    labels = np.asarray(labels, dtype=np.float32)
    mask = np.asarray(label_mask).astype(bool)
    cw = np.asarray(cls_weight, dtype=np.float32)

    gcls = labels[..., 0].astype(np.int32)                      # [B, M]
    gx = labels[..., 1] * IMG
    gy = labels[..., 2] * IMG
    gw = labels[..., 3] * IMG
    gh = labels[..., 4] * IMG
    gi = np.clip(gx / STRIDE, np.float32(0.0), np.float32(W - 0.001)).astype(np.int32)
    gj = np.clip(gy / STRIDE, np.float32(0.0), np.float32(H - 0.001)).astype(np.int32)
    gtw, gth = gw / STRIDE, gh / STRIDE
    ag = ANCHORS / STRIDE                                       # [3, 2]
    inter = np.minimum(gtw[..., None], ag[:, 0]) * np.minimum(gth[..., None], ag[:, 1])
    union = gtw[..., None] * gth[..., None] + ag[:, 0] * ag[:, 1] - inter + np.float32(1e-9)
    best_a = np.argmax(inter / union, axis=-1).astype(np.int32)  # [B, M]

    offs = [(di, dj) for di in (-1, 0, 1) for dj in (-1, 0, 1)]
    # sequential scatter with last-write-wins box, accumulating class set
    targets = {}  # (b, a, j, i) -> [set(cls), (bx, by, bw, bh)]
    for b in range(B):
        for m in range(M):
            if not mask[b, m]:
                continue
            a = int(best_a[b, m])
            c = int(gcls[b, m])
            box = (gx[b, m], gy[b, m], gw[b, m], gh[b, m])
            for di, dj in offs:
                i = min(max(int(gi[b, m]) + di, 0), W - 1)
                j = min(max(int(gj[b, m]) + dj, 0), H - 1)
                e = targets.setdefault((b, a, j, i), [set(), None])
                e[0].add(c)
                e[1] = box
    n_pos = max(len(targets), 1)

    idx_all = np.zeros((NCORES, 128, NGRP), dtype=np.int32)
    meta_all = np.zeros((NCORES, 128, NGRP * NMETA), dtype=np.float32)
    tcls_all = np.zeros((NCORES, 128, NGRP * C), dtype=np.float32)
    wq_all = np.zeros((NCORES, 128, NGRP * C), dtype=np.float32)
    slot_ctr = [0] * NCORES
    for (b, a, j, i), (clsset, box) in targets.items():
        core = b // BL
        s = slot_ctr[core]
        slot_ctr[core] += 1
        assert s < NPOS, "positive-cell capacity exceeded"
        p_, g_ = s % 128, s // 128
        bloc = b - core * BL
        idx_all[core, p_, g_] = ((bloc * NA + a) * H + j) * W + i
        bx, by, bw, bh = box
        gx1 = bx - bw * np.float32(0.5)
        gx2 = bx + bw * np.float32(0.5)
        gy1 = by - bh * np.float32(0.5)
        gy2 = by + bh * np.float32(0.5)
        areag = max(gx2 - gx1, np.float32(0.0)) * max(gy2 - gy1, np.float32(0.0))
        atg = np.float32(np.arctan(bw / (bh + EPS)))
        mslot = np.array(
            [1.0, i * 8.0, j * 8.0, ANCHORS[a, 0], ANCHORS[a, 1],
             bx, by, gx1, gx2, gy1, gy2, areag, atg, 0.0, 0.0, 0.0],
            dtype=np.float32,
        )
        meta_all[core, p_, g_ * NMETA:(g_ + 1) * NMETA] = mslot
        for c in clsset:
            tcls_all[core, p_, g_ * C + c] = 1.0
        wq_all[core, p_, g_ * C:(g_ + 1) * C] = np.float32(0.25) * cw
    return idx_all, meta_all, tcls_all, wq_all, n_pos


def kernel(p_raw, labels, label_mask, cls_weight):
    global LAST_RESULT
    p_raw = np.ascontiguousarray(np.asarray(p_raw, dtype=np.float32))
    idx_all, meta_all, tcls_all, wq_all, n_pos = _assign_targets_host(
        labels, label_mask, cls_weight
    )

    if MODE not in _BUILD_CACHE:
        _BUILD_CACHE[MODE] = _build(MODE)
    nc = _BUILD_CACHE[MODE]

    shards = p_raw.reshape(NCORES, NCELL, CH)
    in_maps = []
    for c in range(NCORES):
        in_maps.append({
            "p": shards[c],
            "idx": idx_all[c],
            "meta": meta_all[c],
            "tcls": tcls_all[c],
            "wq": wq_all[c],
        })

    r = run_bass_kernel_spmd(
        nc, in_maps, core_ids=list(range(NCORES)), trace=TRACE, **TRACE_KW
    )
    LAST_RESULT = r

    outs = np.stack([np.asarray(r.results[c]["out"][0]) for c in range(NCORES)])
    sums = outs.astype(np.float64).sum(axis=0)
    s_dense = sums[:COL_CORR].sum()
    l_obj = 0.25 * (s_dense + sums[COL_CORR]) / float(B * NA * H * W)
    l_box = sums[COL_BOX] / n_pos
    l_cls = sums[COL_CLS] / (n_pos * C)
    total = 7.5 * l_box + 1.0 * l_obj + 0.5 * l_cls
    return np.float32(total)


# revision 9
# speedup vs baseline: 1.1412x; 1.0984x over previous
"""Trainium2 Bass kernel for nn_DBLoss (YOLO-style detection loss).

Strategy (pure data parallel over batch, 8 cores x 4 images):
  * The loss decomposes as 7.5*l_box + l_obj + 0.5*l_cls where only the
    objectness term touches every grid cell; box/cls terms only touch the
    ~180 label-assigned cells per image.
  * Host (numpy) replicates the reference's target assignment on the tiny
    `labels` tensor to produce per-core scatter metadata: positive-cell
    indices, gt-box constants, multi-hot class targets.  Collision
    semantics match the reference scatter: tbox last-write-wins, tcls
    accumulates classes (class is part of the scatter index).
  * Device: streams the p_raw shard to compute sum(focal_bce(obj_logit, 0))
    over all cells, gathers positive cells by indirect DMA, computes the
    obj t=1 correction, CIoU box loss and weighted focal cls loss there,
    and emits per-core partial sums.
  * Host sums 8x16 partials and applies the n_pos / mean normalizations.

All transcendentals use only the Exp and Ln ACT LUTs (one table set:
natural_log_exp_and_others), so a single act-table load suffices:
  softplus(x)        = ln(1 + exp(x))            (clamped at 88)
  sigmoid(x)^1.5     = exp(-1.5 * softplus(-x))
  (1-sigmoid(x))^1.5 = exp(-1.5 * softplus(x))
  sigmoid(x)         = 1/(1 + exp(-x))           (DVE reciprocal is exact)
  u^1.5              = exp(1.5 * ln(max(u, tiny)))
  arctan             = odd polynomial in z^2 after range reduction (DVE)
"""

import sys

sys.path.insert(0, "/opt/trn_rl_repo")

import numpy as np

import concourse.bass as bass
import concourse.tile as tile
from concourse import mybir
from concourse.bass import IndirectOffsetOnAxis
from concourse.bass_utils import run_bass_kernel_spmd

f32 = mybir.dt.float32
i32 = mybir.dt.int32
AF = mybir.ActivationFunctionType
ALU = mybir.AluOpType
AX = mybir.AxisListType

# problem constants (hardcoded per harness contract)
B, NA, H, W, M, C = 32, 3, 80, 80, 20, 80
CH = 5 + C
NCORES = 8
BL = B // NCORES            # 4 images per core
NCELL = BL * NA * H * W     # 76800 cells per core
NGRP = 6                    # positive-cell capacity = 6*128 = 768 >= 4*20*9
NPOS = NGRP * 128
NMETA = 16                  # f32 slots per positive cell
STRIDE = np.float32(8.0)
IMG = np.float32(640.0)
EPS = np.float32(1e-7)
PI2 = np.float32(np.pi ** 2)
ANCHORS = np.array([[10.0, 13.0], [16.0, 30.0], [33.0, 23.0]], dtype=np.float32)

# atan(z)/z ~ poly(z^2) on [0,1], max err ~6e-7 (f32 horner)
ATAN_C = [0.9999993278352405, -0.33326374521881663, 0.1987987215570962,
          -0.1348040560754345, 0.08374155654506504, -0.03689862924626238,
          0.007825482945513086]

# streaming config (full mode): NT tiles of [128 partitions x KC cells]
NT = 12
KC = NCELL // (NT * 128)    # 50 cells/partition/tile
NTS = 4                     # strided mode: 4 tiles of [128 x 150]
KS = NCELL // (NTS * 128)

# partial-sum column map (out[0, k])
COL_CORR, COL_BOX, COL_CLS, NCOL = 12, 13, 14, 16

MODE = "full"               # "full" (stream all of p_raw) or "strided" (ch4 only)
TRACE = False
TRACE_KW = {}
LAST_RESULT = None

_BUILD_CACHE = {}


def _split_multi_waits(nc, limit=1):
    """This container's walrus build accepts only one sync-wait per
    instruction; split Tile's stacked waits into single-wait NoOp chains."""
    n = 0
    for fn in nc.m.functions:
        for bb in fn.blocks:
            new_insts, changed = [], False
            for inst in bb.instructions:
                si = getattr(inst, "sync_info", None)
                waits = list(si.on_wait) if si is not None and si.on_wait else []
                if len(waits) > limit:
                    changed = True
                    n += 1
                    for w in waits[:-limit]:
                        nop = mybir.InstNoOp(
                            name=nc.get_next_instruction_name(),
                            engine=inst.engine,
                            sync_info=mybir.SyncInfo(on_wait=[w], on_update=[]),
                            bass_nofuse=True,
                        )
                        nc.register_instruction(nop)
                        new_insts.append(nop)
                    si.on_wait = waits[-limit:]
                new_insts.append(inst)
            if changed:
                try:
                    bb.instructions = new_insts
                except Exception:
                    bb.instructions[:] = new_insts
    return n


def _build(mode):
    nc = bass.Bass()
    p = nc.declare_dram_parameter("p", [NCELL, CH], f32, isOutput=False)
    idx = nc.declare_dram_parameter("idx", [128, NGRP], i32, isOutput=False)
    meta = nc.declare_dram_parameter("meta", [128, NGRP * NMETA], f32, isOutput=False)
    tcls = nc.declare_dram_parameter("tcls", [128, NGRP * C], f32, isOutput=False)
    wq = nc.declare_dram_parameter("wq", [128, NGRP * C], f32, isOutput=False)
    outp = nc.declare_dram_parameter("out", [1, NCOL], f32, isOutput=True)

    with tile.TileContext(nc) as tc:
        with tc.tile_pool(name="stream", bufs=3) as streamp, \
             tc.tile_pool(name="work", bufs=2) as workp, \
             tc.tile_pool(name="small", bufs=1) as smallp, \
             tc.tile_pool(name="psum", bufs=1, space="PSUM") as psump:

            partials = smallp.tile([128, NCOL], f32)
            nc.vector.memset(partials[:], 0.0)

            # ---------------- dense objectness pass ----------------
            # focal_bce(x, 0) = 0.25 * exp(-1.5*softplus(-x)) * softplus(x)
            def obj_dense(x_ap, n, col):
                shp = [128] + (n if isinstance(n, list) else [n])
                e = workp.tile(shp, f32, tag="e", name="e")
                l = workp.tile(shp, f32, tag="l", name="l")
                spn = workp.tile(shp, f32, tag="spn", name="spn")
                g = workp.tile(shp, f32, tag="g", name="g")
                sc = workp.tile(shp, f32, tag="sc", name="sc")
                nc.scalar.activation(e[:], x_ap, AF.Exp)             # e^x
                nc.scalar.activation(l[:], e[:], AF.Ln, bias=1.0)    # softplus(x)
                nc.vector.tensor_scalar_min(l[:], l[:], 88.0)
                nc.vector.tensor_sub(spn[:], l[:], x_ap)             # softplus(-x)
                nc.scalar.activation(g[:], spn[:], AF.Exp, scale=-1.5)
                nc.vector.tensor_mul(sc[:], g[:], l[:])
                ax = AX.XY if isinstance(n, list) else AX.X
                nc.vector.tensor_reduce(
                    out=partials[:, col:col + 1], in_=sc[:],
                    axis=ax, op=ALU.add,
                )

            # ---------------- positive-cell pass ----------------
            idx_t = smallp.tile([128, NGRP], i32)
            nc.gpsimd.dma_start(out=idx_t[:], in_=idx[:])
            meta_t = smallp.tile([128, NGRP * NMETA], f32)
            nc.gpsimd.dma_start(out=meta_t[:], in_=meta[:])
            tcls_t = smallp.tile([128, NGRP * C], f32)
            nc.gpsimd.dma_start(out=tcls_t[:], in_=tcls[:])
            wq_t = smallp.tile([128, NGRP * C], f32)
            nc.gpsimd.dma_start(out=wq_t[:], in_=wq[:])

            pos = smallp.tile([128, NGRP * CH], f32)
            pos3 = pos[:].rearrange("p (g c) -> p g c", c=CH)
            for g_ in range(NGRP):
                nc.gpsimd.indirect_dma_start(
                    out=pos3[:, g_, :],
                    out_offset=None,
                    in_=p[:],
                    in_offset=IndirectOffsetOnAxis(ap=idx_t[:, g_:g_ + 1], axis=0),
                )

            m3 = meta_t[:].rearrange("p (g k) -> p g k", k=NMETA)

            def mk(k):
                return m3[:, :, k]

            valid, cx8, cy8, awpx, ahpx = mk(0), mk(1), mk(2), mk(3), mk(4)
            gxm, gym = mk(5), mk(6)
            gx1, gx2, gy1, gy2 = mk(7), mk(8), mk(9), mk(10)
            areag, atg = mk(11), mk(12)

            G = [128, NGRP]

            def t6(tag):
                return workp.tile(G, f32, tag=tag, name=tag)

            # --- objectness correction at positive cells: t goes 0 -> 1 ---
            xo = pos3[:, :, 4]
            eo, lo, spn6 = t6("eo"), t6("lo"), t6("spn6")
            g0, g1, sc6 = t6("g0"), t6("g1"), t6("sc6")
            nc.scalar.activation(eo[:], xo, AF.Exp)
            nc.scalar.activation(lo[:], eo[:], AF.Ln, bias=1.0)
            nc.vector.tensor_scalar_min(lo[:], lo[:], 88.0)          # softplus(x)
            nc.vector.tensor_sub(spn6[:], lo[:], xo)                 # softplus(-x)
            nc.scalar.activation(g0[:], spn6[:], AF.Exp, scale=-1.5)  # s^1.5
            nc.scalar.activation(g1[:], lo[:], AF.Exp, scale=-1.5)   # (1-s)^1.5
            nc.vector.tensor_mul(g0[:], g0[:], lo[:])                # f0/alpha
            nc.vector.tensor_mul(g1[:], g1[:], spn6[:])              # f1/alpha
            nc.vector.tensor_sub(g1[:], g1[:], g0[:])
            nc.vector.tensor_mul(sc6[:], g1[:], valid)
            nc.vector.tensor_reduce(
                out=partials[:, COL_CORR:COL_CORR + 1], in_=sc6[:],
                axis=AX.X, op=ALU.add,
            )

            # --- CIoU box loss at positive cells ---
            sx, sy, pw, ph = t6("sx"), t6("sy"), t6("pw"), t6("ph")
            nc.scalar.activation(sx[:], pos3[:, :, 0], AF.Exp, scale=-1.0)
            nc.vector.tensor_scalar_add(sx[:], sx[:], 1.0)
            nc.vector.reciprocal(sx[:], sx[:])                       # sigmoid(x0)
            nc.scalar.activation(sy[:], pos3[:, :, 1], AF.Exp, scale=-1.0)
            nc.vector.tensor_scalar_add(sy[:], sy[:], 1.0)
            nc.vector.reciprocal(sy[:], sy[:])                       # sigmoid(x1)
            nc.scalar.activation(pw[:], pos3[:, :, 2], AF.Exp)
            nc.scalar.activation(ph[:], pos3[:, :, 3], AF.Exp)
            px, py = t6("px"), t6("py")
            nc.vector.scalar_tensor_tensor(
                out=px[:], in0=sx[:], scalar=8.0, in1=cx8, op0=ALU.mult, op1=ALU.add)
            nc.vector.scalar_tensor_tensor(
                out=py[:], in0=sy[:], scalar=8.0, in1=cy8, op0=ALU.mult, op1=ALU.add)
            nc.vector.tensor_mul(pw[:], pw[:], awpx)
            nc.vector.tensor_mul(ph[:], ph[:], ahpx)
            px1, px2, py1, py2 = t6("px1"), t6("px2"), t6("py1"), t6("py2")
            hw, hh = t6("hw"), t6("hh")
            nc.vector.tensor_scalar_mul(hw[:], pw[:], 0.5)
            nc.vector.tensor_scalar_mul(hh[:], ph[:], 0.5)
            nc.vector.tensor_sub(px1[:], px[:], hw[:])
            nc.vector.tensor_add(px2[:], px[:], hw[:])
            nc.vector.tensor_sub(py1[:], py[:], hh[:])
            nc.vector.tensor_add(py2[:], py[:], hh[:])
            a6, b6, iw, ih = t6("a6"), t6("b6"), t6("iw"), t6("ih")
            nc.vector.tensor_tensor(out=a6[:], in0=px2[:], in1=gx2, op=ALU.min)
            nc.vector.tensor_tensor(out=b6[:], in0=px1[:], in1=gx1, op=ALU.max)
            nc.vector.tensor_sub(iw[:], a6[:], b6[:])
            nc.vector.tensor_scalar_max(iw[:], iw[:], 0.0)
            nc.vector.tensor_tensor(out=a6[:], in0=py2[:], in1=gy2, op=ALU.min)
            nc.vector.tensor_tensor(out=b6[:], in0=py1[:], in1=gy1, op=ALU.max)
            nc.vector.tensor_sub(ih[:], a6[:], b6[:])
            nc.vector.tensor_scalar_max(ih[:], ih[:], 0.0)
            inter = t6("inter")
            nc.vector.tensor_mul(inter[:], iw[:], ih[:])
            # union = relu(px2-px1)*relu(py2-py1) + areag - inter + EPS
            ap_, bp_ = t6("ap_"), t6("bp_")
            nc.vector.tensor_sub(ap_[:], px2[:], px1[:])
            nc.vector.tensor_scalar_max(ap_[:], ap_[:], 0.0)
            nc.vector.tensor_sub(bp_[:], py2[:], py1[:])
            nc.vector.tensor_scalar_max(bp_[:], bp_[:], 0.0)
            union = t6("union")
            nc.vector.tensor_mul(union[:], ap_[:], bp_[:])
            nc.vector.tensor_add(union[:], union[:], areag)
            nc.vector.tensor_sub(union[:], union[:], inter[:])
            nc.vector.tensor_scalar_add(union[:], union[:], float(EPS))
            iou = t6("iou")
            nc.vector.reciprocal(iou[:], union[:])
            nc.vector.tensor_mul(iou[:], inter[:], iou[:])
            # enclosing box diag^2
            cw, chv = t6("cw"), t6("chv")
            nc.vector.tensor_tensor(out=a6[:], in0=px2[:], in1=gx2, op=ALU.max)
            nc.vector.tensor_tensor(out=b6[:], in0=px1[:], in1=gx1, op=ALU.min)
            nc.vector.tensor_sub(cw[:], a6[:], b6[:])
            nc.vector.tensor_scalar_max(cw[:], cw[:], 0.0)
            nc.vector.tensor_tensor(out=a6[:], in0=py2[:], in1=gy2, op=ALU.max)
            nc.vector.tensor_tensor(out=b6[:], in0=py1[:], in1=gy1, op=ALU.min)
            nc.vector.tensor_sub(chv[:], a6[:], b6[:])
            nc.vector.tensor_scalar_max(chv[:], chv[:], 0.0)
            c2 = t6("c2")
            nc.vector.tensor_mul(cw[:], cw[:], cw[:])
            nc.vector.tensor_mul(chv[:], chv[:], chv[:])
            nc.vector.tensor_add(c2[:], cw[:], chv[:])
            nc.vector.tensor_scalar_add(c2[:], c2[:], float(EPS))
            rho2 = t6("rho2")
            nc.vector.tensor_tensor(out=a6[:], in0=px[:], in1=gxm, op=ALU.subtract)
            nc.vector.tensor_mul(a6[:], a6[:], a6[:])
            nc.vector.tensor_tensor(out=b6[:], in0=py[:], in1=gym, op=ALU.subtract)
            nc.vector.tensor_mul(b6[:], b6[:], b6[:])
            nc.vector.tensor_add(rho2[:], a6[:], b6[:])
            # atan(pw/(ph+EPS)) via polynomial (no trig table)
            q, qi, z, z2 = t6("q"), t6("qi"), t6("z"), t6("z2")
            nc.vector.tensor_scalar_add(q[:], ph[:], float(EPS))
            nc.vector.reciprocal(q[:], q[:])
            nc.vector.tensor_mul(q[:], pw[:], q[:])                  # q > 0
            nc.vector.reciprocal(qi[:], q[:])
            nc.vector.tensor_tensor(out=z[:], in0=q[:], in1=qi[:], op=ALU.min)
            nc.vector.tensor_mul(z2[:], z[:], z[:])
            acc = t6("acc")
            nc.vector.tensor_scalar(
                out=acc[:], in0=z2[:], scalar1=float(ATAN_C[6]),
                scalar2=float(ATAN_C[5]), op0=ALU.mult, op1=ALU.add)
            for k in (4, 3, 2, 1, 0):
                nc.vector.tensor_mul(acc[:], acc[:], z2[:])
                nc.vector.tensor_scalar_add(acc[:], acc[:], float(ATAN_C[k]))
            nc.vector.tensor_mul(acc[:], acc[:], z[:])               # atan(z)
            flag = t6("flag")
            nc.vector.tensor_scalar(
                out=flag[:], in0=q[:], scalar1=1.0, scalar2=None, op0=ALU.is_gt)
            fw = t6("fw")
            nc.vector.tensor_scalar(
                out=fw[:], in0=acc[:], scalar1=-2.0,
                scalar2=float(np.pi / 2), op0=ALU.mult, op1=ALU.add)
            nc.vector.tensor_mul(fw[:], fw[:], flag[:])
            nc.vector.tensor_add(acc[:], acc[:], fw[:])              # atan(q)
            vv = t6("vv")
            nc.vector.tensor_tensor(out=vv[:], in0=atg, in1=acc[:], op=ALU.subtract)
            nc.vector.tensor_mul(vv[:], vv[:], vv[:])
            nc.vector.tensor_scalar_mul(vv[:], vv[:], float(np.float32(4.0) / PI2))
            # alpha = v / (1 - iou + v + EPS)
            den = t6("den")
            nc.vector.scalar_tensor_tensor(
                out=den[:], in0=iou[:], scalar=-1.0, in1=vv[:],
                op0=ALU.mult, op1=ALU.add)
            nc.vector.tensor_scalar_add(den[:], den[:], float(1.0 + float(EPS)))
            nc.vector.reciprocal(den[:], den[:])
            nc.vector.tensor_mul(den[:], vv[:], den[:])              # alpha
            nc.vector.tensor_mul(den[:], den[:], vv[:])              # alpha*v
            # loss = 1 - iou + rho2/c2 + alpha*v
            nc.vector.reciprocal(c2[:], c2[:])
            nc.vector.tensor_mul(rho2[:], rho2[:], c2[:])
            nc.vector.tensor_add(den[:], den[:], rho2[:])
            nc.vector.tensor_sub(den[:], den[:], iou[:])
            nc.vector.tensor_scalar_add(den[:], den[:], 1.0)
            bsc = t6("bsc")
            nc.vector.tensor_mul(bsc[:], den[:], valid)
            nc.vector.tensor_reduce(
                out=partials[:, COL_BOX:COL_BOX + 1], in_=bsc[:],
                axis=AX.X, op=ALU.add,
            )

            # --- weighted focal class loss at positive cells ---
            NCL = NGRP * C
            xc = pos3[:, :, 5:]                                      # [128,6,80]

            def tcl(name):
                return smallp.tile([128, NCL], f32, name=name)

            ecl, scl, lcl = tcl("ecl"), tcl("scl"), tcl("lcl")
            ucl, fcl, sccl = tcl("ucl"), tcl("fcl"), tcl("sccl")
            e3 = ecl[:].rearrange("p (g c) -> p g c", c=C)
            nc.scalar.activation(e3, xc, AF.Exp)                     # e^x
            nc.vector.tensor_scalar_add(scl[:], ecl[:], 1.0)
            nc.vector.reciprocal(scl[:], scl[:])                     # 1 - sigmoid
            nc.vector.tensor_scalar(
                out=scl[:], in0=scl[:], scalar1=-1.0, scalar2=1.0,
                op0=ALU.mult, op1=ALU.add)                           # sigmoid
            nc.scalar.activation(lcl[:], ecl[:], AF.Ln, bias=1.0)    # softplus
            nc.vector.tensor_scalar_min(lcl[:], lcl[:], 88.0)
            nc.vector.tensor_mul(ucl[:], scl[:], tcls_t[:])          # s*t
            nc.vector.scalar_tensor_tensor(
                out=ucl[:], in0=ucl[:], scalar=-2.0, in1=scl[:],
                op0=ALU.mult, op1=ALU.add)                           # s - 2st
            nc.vector.tensor_add(ucl[:], ucl[:], tcls_t[:])          # u
            nc.vector.tensor_scalar_max(ucl[:], ucl[:], 1e-38)
            nc.scalar.activation(ucl[:], ucl[:], AF.Ln)
            nc.scalar.activation(ucl[:], ucl[:], AF.Exp, scale=1.5)  # u^1.5
            f3 = fcl[:].rearrange("p (g c) -> p g c", c=C)
            nc.vector.tensor_tensor(out=f3, in0=xc, in1=tcls_t[:].rearrange(
                "p (g c) -> p g c", c=C), op=ALU.mult)               # x*t
            nc.vector.tensor_sub(fcl[:], lcl[:], fcl[:])             # bce
            nc.vector.tensor_mul(fcl[:], ucl[:], fcl[:])
            nc.vector.tensor_mul(sccl[:], fcl[:], wq_t[:])
            nc.vector.tensor_reduce(
                out=partials[:, COL_CLS:COL_CLS + 1], in_=sccl[:],
                axis=AX.X, op=ALU.add,
            )


            if mode == "full":
                pt = p[:].rearrange("(t p k) c -> t p (k c)", t=NT, p=128)
                for t in range(NT):
                    xt = streamp.tile([128, KC * CH], f32, tag="xt", name="xt")
                    nc.sync.dma_start(out=xt[:], in_=pt[t])
                    ch4 = xt[:].rearrange("p (k c) -> p k c", c=CH)[:, :, 4]
                    obj_dense(ch4, KC, t)
            elif mode == "pair":
                # one descriptor spans ch4 of two adjacent cells (86 floats):
                # halves descriptor count; engines move 344B instead of 2x4B
                NPAIR = NCELL // 2           # 38400
                NTP = 6
                KP = NPAIR // (NTP * 128)    # 50 pairs/partition/tile
                for t in range(NTP):
                    xt = streamp.tile([128, KP * 86], f32, tag="xp", name="xp")
                    src = bass.AP(
                        tensor=p[:].tensor,
                        offset=4 + t * (128 * KP) * 170,
                        ap=[[170 * KP, 128], [170, KP], [1, 86]],
                    )
                    eng = nc.sync if t % 2 == 0 else nc.scalar
                    eng.dma_start(out=xt[:].rearrange(
                        "q (k c) -> q k c", c=86), in_=src)
                    ch4 = xt[:].rearrange("q (k c) -> q k c", c=86)[:, :, 0:86:85]
                    obj_dense(ch4, [KP, 2], t)
            elif mode == "strided":
                ps4 = p[:].rearrange("(t p k) c -> t p k c", t=NTS, p=128)
                for t in range(NTS):
                    xt = streamp.tile([128, KS], f32, tag="xs", name="xs")
                    nc.sync.dma_start(out=xt[:], in_=ps4[t, :, :, 4])
                    obj_dense(xt[:], KS, t)
            else:  # strided2: split ch4 extraction over both HWDGE rings
                NT2 = 8
                K2 = NCELL // (NT2 * 128)
                ps8 = p[:].rearrange("(t p k) c -> t p k c", t=NT2, p=128)
                for t in range(NT2):
                    xt = streamp.tile([128, K2], f32, tag="xs2", name="xs2")
                    eng = nc.sync if t % 2 == 0 else nc.scalar
                    eng.dma_start(out=xt[:], in_=ps8[t, :, :, 4])
                    obj_dense(xt[:], K2, t)

            # ---------------- cross-partition reduce + store ----------------
            ones = smallp.tile([128, 1], f32)
            nc.vector.memset(ones[:], 1.0)
            ps = psump.tile([1, NCOL], f32)
            nc.tensor.matmul(out=ps[:], lhsT=ones[:], rhs=partials[:],
                             start=True, stop=True)
            res = smallp.tile([1, NCOL], f32)
            nc.vector.tensor_copy(out=res[:], in_=ps[:])
            nc.sync.dma_start(out=outp[:], in_=res[:])

    _split_multi_waits(nc)
    return nc


def _assign_targets_host(labels, label_mask, cls_weight):
    """Replicate reference.assign_targets scatter on host; returns per-core
    device aux inputs and global n_pos."""
    labels = np.asarray(labels, dtype=np.float32)
    mask = np.asarray(label_mask).astype(bool)
    cw = np.asarray(cls_weight, dtype=np.float32)

    gcls = labels[..., 0].astype(np.int32)                      # [B, M]
    gx = labels[..., 1] * IMG
    gy = labels[..., 2] * IMG
    gw = labels[..., 3] * IMG
    gh = labels[..., 4] * IMG
    gi = np.clip(gx / STRIDE, np.float32(0.0), np.float32(W - 0.001)).astype(np.int32)
    gj = np.clip(gy / STRIDE, np.float32(0.0), np.float32(H - 0.001)).astype(np.int32)
    gtw, gth = gw / STRIDE, gh / STRIDE
    ag = ANCHORS / STRIDE                                       # [3, 2]
    inter = np.minimum(gtw[..., None], ag[:, 0]) * np.minimum(gth[..., None], ag[:, 1])
    union = gtw[..., None] * gth[..., None] + ag[:, 0] * ag[:, 1] - inter + np.float32(1e-9)
    best_a = np.argmax(inter / union, axis=-1).astype(np.int32)  # [B, M]

    offs = [(di, dj) for di in (-1, 0, 1) for dj in (-1, 0, 1)]
    # sequential scatter with last-write-wins box, accumulating class set
    targets = {}  # (b, a, j, i) -> [set(cls), (bx, by, bw, bh)]
    for b in range(B):
        for m in range(M):
            if not mask[b, m]:
                continue
            a = int(best_a[b, m])
            c = int(gcls[b, m])
            box = (gx[b, m], gy[b, m], gw[b, m], gh[b, m])
            for di, dj in offs:
                i = min(max(int(gi[b, m]) + di, 0), W - 1)
                j = min(max(int(gj[b, m]) + dj, 0), H - 1)
                e = targets.setdefault((b, a, j, i), [set(), None])
                e[0].add(c)
                e[1] = box
    n_pos = max(len(targets), 1)

    idx_all = np.zeros((NCORES, 128, NGRP), dtype=np.int32)
    meta_all = np.zeros((NCORES, 128, NGRP * NMETA), dtype=np.float32)
    tcls_all = np.zeros((NCORES, 128, NGRP * C), dtype=np.float32)
    wq_all = np.zeros((NCORES, 128, NGRP * C), dtype=np.float32)
    slot_ctr = [0] * NCORES
    for (b, a, j, i), (clsset, box) in targets.items():
        core = b // BL
        s = slot_ctr[core]
        slot_ctr[core] += 1
        assert s < NPOS, "positive-cell capacity exceeded"
        p_, g_ = s % 128, s // 128
        bloc = b - core * BL
        idx_all[core, p_, g_] = ((bloc * NA + a) * H + j) * W + i
        bx, by, bw, bh = box
        gx1 = bx - bw * np.float32(0.5)
        gx2 = bx + bw * np.float32(0.5)
        gy1 = by - bh * np.float32(0.5)
        gy2 = by + bh * np.float32(0.5)
        areag = max(gx2 - gx1, np.float32(0.0)) * max(gy2 - gy1, np.float32(0.0))
        atg = np.float32(np.arctan(bw / (bh + EPS)))
        mslot = np.array(
            [1.0, i * 8.0, j * 8.0, ANCHORS[a, 0], ANCHORS[a, 1],
             bx, by, gx1, gx2, gy1, gy2, areag, atg, 0.0, 0.0, 0.0],
            dtype=np.float32,
        )
        meta_all[core, p_, g_ * NMETA:(g_ + 1) * NMETA] = mslot
        for c in clsset:
            tcls_all[core, p_, g_ * C + c] = 1.0
        wq_all[core, p_, g_ * C:(g_ + 1) * C] = np.float32(0.25) * cw
    return idx_all, meta_all, tcls_all, wq_all, n_pos


def kernel(p_raw, labels, label_mask, cls_weight):
    global LAST_RESULT
    p_raw = np.ascontiguousarray(np.asarray(p_raw, dtype=np.float32))
    idx_all, meta_all, tcls_all, wq_all, n_pos = _assign_targets_host(
        labels, label_mask, cls_weight
    )

    if MODE not in _BUILD_CACHE:
        _BUILD_CACHE[MODE] = _build(MODE)
    nc = _BUILD_CACHE[MODE]

    shards = p_raw.reshape(NCORES, NCELL, CH)
    in_maps = []
    for c in range(NCORES):
        in_maps.append({
            "p": shards[c],
            "idx": idx_all[c],
            "meta": meta_all[c],
            "tcls": tcls_all[c],
            "wq": wq_all[c],
        })

    r = run_bass_kernel_spmd(
        nc, in_maps, core_ids=list(range(NCORES)), trace=TRACE, **TRACE_KW
    )
    LAST_RESULT = r

    outs = np.stack([np.asarray(r.results[c]["out"][0]) for c in range(NCORES)])
    sums = outs.astype(np.float64).sum(axis=0)
    s_dense = sums[:COL_CORR].sum()
    l_obj = 0.25 * (s_dense + sums[COL_CORR]) / float(B * NA * H * W)
    l_box = sums[COL_BOX] / n_pos
    l_cls = sums[COL_CLS] / (n_pos * C)
    total = 7.5 * l_box + 1.0 * l_obj + 0.5 * l_cls
    return np.float32(total)


# revision 10
# speedup vs baseline: 1.1437x; 1.0021x over previous
"""Trainium2 Bass kernel for nn_DBLoss (YOLO-style detection loss).

Strategy (pure data parallel over batch, 8 cores x 4 images):
  * The loss decomposes as 7.5*l_box + l_obj + 0.5*l_cls where only the
    objectness term touches every grid cell; box/cls terms only touch the
    ~180 label-assigned cells per image.
  * Host (numpy) replicates the reference's target assignment on the tiny
    `labels` tensor to produce per-core scatter metadata: positive-cell
    indices, gt-box constants, multi-hot class targets.  Collision
    semantics match the reference scatter: tbox last-write-wins, tcls
    accumulates classes (class is part of the scatter index).
  * Device: streams the p_raw shard to compute sum(focal_bce(obj_logit, 0))
    over all cells, gathers positive cells by indirect DMA, computes the
    obj t=1 correction, CIoU box loss and weighted focal cls loss there,
    and emits per-core partial sums.
  * Host sums 8x16 partials and applies the n_pos / mean normalizations.

All transcendentals use only the Exp and Ln ACT LUTs (one table set:
natural_log_exp_and_others), so a single act-table load suffices:
  softplus(x)        = ln(1 + exp(x))            (clamped at 88)
  sigmoid(x)^1.5     = exp(-1.5 * softplus(-x))
  (1-sigmoid(x))^1.5 = exp(-1.5 * softplus(x))
  sigmoid(x)         = 1/(1 + exp(-x))           (DVE reciprocal is exact)
  u^1.5              = exp(1.5 * ln(max(u, tiny)))
  arctan             = odd polynomial in z^2 after range reduction (DVE)
"""

import sys

sys.path.insert(0, "/opt/trn_rl_repo")

import numpy as np

import concourse.bass as bass
import concourse.tile as tile
from concourse import mybir
from concourse.bass import IndirectOffsetOnAxis
from concourse.bass_utils import run_bass_kernel_spmd

f32 = mybir.dt.float32
i32 = mybir.dt.int32
AF = mybir.ActivationFunctionType
ALU = mybir.AluOpType
AX = mybir.AxisListType

# problem constants (hardcoded per harness contract)
B, NA, H, W, M, C = 32, 3, 80, 80, 20, 80
CH = 5 + C
NCORES = 8
BL = B // NCORES            # 4 images per core
NCELL = BL * NA * H * W     # 76800 cells per core
NGRP = 6                    # positive-cell capacity = 6*128 = 768 >= 4*20*9
NPOS = NGRP * 128
NMETA = 16                  # f32 slots per positive cell
STRIDE = np.float32(8.0)
IMG = np.float32(640.0)
EPS = np.float32(1e-7)
PI2 = np.float32(np.pi ** 2)
ANCHORS = np.array([[10.0, 13.0], [16.0, 30.0], [33.0, 23.0]], dtype=np.float32)

# atan(z)/z ~ poly(z^2) on [0,1], max err ~6e-7 (f32 horner)
ATAN_C = [0.9999993278352405, -0.33326374521881663, 0.1987987215570962,
          -0.1348040560754345, 0.08374155654506504, -0.03689862924626238,
          0.007825482945513086]

# streaming config (full mode): NT tiles of [128 partitions x KC cells]
NT = 12
KC = NCELL // (NT * 128)    # 50 cells/partition/tile
NTS = 4                     # strided mode: 4 tiles of [128 x 150]
KS = NCELL // (NTS * 128)

# partial-sum column map (out[0, k])
COL_CORR, COL_BOX, COL_CLS, NCOL = 12, 13, 14, 16

MODE = "full"               # "full" (stream all of p_raw) or "strided" (ch4 only)
TRACE = False
TRACE_KW = {}
LAST_RESULT = None

_BUILD_CACHE = {}


def _split_multi_waits(nc, limit=1):
    """This container's walrus build accepts only one sync-wait per
    instruction; split Tile's stacked waits into single-wait NoOp chains."""
    n = 0
    for fn in nc.m.functions:
        for bb in fn.blocks:
            new_insts, changed = [], False
            for inst in bb.instructions:
                si = getattr(inst, "sync_info", None)
                waits = list(si.on_wait) if si is not None and si.on_wait else []
                if len(waits) > limit:
                    changed = True
                    n += 1
                    for w in waits[:-limit]:
                        nop = mybir.InstNoOp(
                            name=nc.get_next_instruction_name(),
                            engine=inst.engine,
                            sync_info=mybir.SyncInfo(on_wait=[w], on_update=[]),
                            bass_nofuse=True,
                        )
                        nc.register_instruction(nop)
                        new_insts.append(nop)
                    si.on_wait = waits[-limit:]
                new_insts.append(inst)
            if changed:
                try:
                    bb.instructions = new_insts
                except Exception:
                    bb.instructions[:] = new_insts
    return n


def _build(mode):
    nc = bass.Bass()
    p = nc.declare_dram_parameter("p", [NCELL, CH], f32, isOutput=False)
    idx = nc.declare_dram_parameter("idx", [128, NGRP], i32, isOutput=False)
    meta = nc.declare_dram_parameter("meta", [128, NGRP * NMETA], f32, isOutput=False)
    tcls = nc.declare_dram_parameter("tcls", [128, NGRP * C], f32, isOutput=False)
    wq = nc.declare_dram_parameter("wq", [128, NGRP * C], f32, isOutput=False)
    outp = nc.declare_dram_parameter("out", [1, NCOL], f32, isOutput=True)

    with tile.TileContext(nc) as tc:
        with tc.tile_pool(name="stream", bufs=3) as streamp, \
             tc.tile_pool(name="work", bufs=2) as workp, \
             tc.tile_pool(name="small", bufs=1) as smallp, \
             tc.tile_pool(name="psum", bufs=1, space="PSUM") as psump:

            partials = smallp.tile([128, NCOL], f32)
            nc.vector.memset(partials[:], 0.0)

            # ---------------- dense objectness pass ----------------
            # focal_bce(x, 0) = 0.25 * exp(-1.5*softplus(-x)) * softplus(x)
            def obj_dense(x_ap, n, col):
                shp = [128] + (n if isinstance(n, list) else [n])
                e = workp.tile(shp, f32, tag="e", name="e")
                l = workp.tile(shp, f32, tag="l", name="l")
                spn = workp.tile(shp, f32, tag="spn", name="spn")
                g = workp.tile(shp, f32, tag="g", name="g")
                sc = workp.tile(shp, f32, tag="sc", name="sc")
                nc.scalar.activation(e[:], x_ap, AF.Exp)             # e^x
                nc.scalar.activation(l[:], e[:], AF.Ln, bias=1.0)    # softplus(x)
                nc.vector.tensor_scalar_min(l[:], l[:], 88.0)
                nc.vector.tensor_sub(spn[:], l[:], x_ap)             # softplus(-x)
                nc.scalar.activation(g[:], spn[:], AF.Exp, scale=-1.5)
                nc.vector.tensor_mul(sc[:], g[:], l[:])
                ax = AX.XY if isinstance(n, list) else AX.X
                nc.vector.tensor_reduce(
                    out=partials[:, col:col + 1], in_=sc[:],
                    axis=ax, op=ALU.add,
                )

            # ---------------- positive-cell pass ----------------
            idx_t = smallp.tile([128, NGRP], i32)
            nc.gpsimd.dma_start(out=idx_t[:], in_=idx[:])
            meta_t = smallp.tile([128, NGRP * NMETA], f32)
            nc.gpsimd.dma_start(out=meta_t[:], in_=meta[:])
            tcls_t = smallp.tile([128, NGRP * C], f32)
            nc.gpsimd.dma_start(out=tcls_t[:], in_=tcls[:])
            wq_t = smallp.tile([128, NGRP * C], f32)
            nc.gpsimd.dma_start(out=wq_t[:], in_=wq[:])

            pos = smallp.tile([128, NGRP * CH], f32)
            pos3 = pos[:].rearrange("p (g c) -> p g c", c=CH)
            for g_ in range(NGRP):
                nc.gpsimd.indirect_dma_start(
                    out=pos3[:, g_, :],
                    out_offset=None,
                    in_=p[:],
                    in_offset=IndirectOffsetOnAxis(ap=idx_t[:, g_:g_ + 1], axis=0),
                )

            m3 = meta_t[:].rearrange("p (g k) -> p g k", k=NMETA)

            def mk(k):
                return m3[:, :, k]

            valid, cx8, cy8, awpx, ahpx = mk(0), mk(1), mk(2), mk(3), mk(4)
            gxm, gym = mk(5), mk(6)
            gx1, gx2, gy1, gy2 = mk(7), mk(8), mk(9), mk(10)
            areag, atg = mk(11), mk(12)

            G = [128, NGRP]

            def t6(tag):
                return workp.tile(G, f32, tag=tag, name=tag)

            # --- objectness correction at positive cells: t goes 0 -> 1 ---
            xo = pos3[:, :, 4]
            eo, lo, spn6 = t6("eo"), t6("lo"), t6("spn6")
            g0, g1, sc6 = t6("g0"), t6("g1"), t6("sc6")
            nc.scalar.activation(eo[:], xo, AF.Exp)
            nc.scalar.activation(lo[:], eo[:], AF.Ln, bias=1.0)
            nc.vector.tensor_scalar_min(lo[:], lo[:], 88.0)          # softplus(x)
            nc.vector.tensor_sub(spn6[:], lo[:], xo)                 # softplus(-x)
            nc.scalar.activation(g0[:], spn6[:], AF.Exp, scale=-1.5)  # s^1.5
            nc.scalar.activation(g1[:], lo[:], AF.Exp, scale=-1.5)   # (1-s)^1.5
            nc.vector.tensor_mul(g0[:], g0[:], lo[:])                # f0/alpha
            nc.vector.tensor_mul(g1[:], g1[:], spn6[:])              # f1/alpha
            nc.vector.tensor_sub(g1[:], g1[:], g0[:])
            nc.vector.tensor_mul(sc6[:], g1[:], valid)
            nc.vector.tensor_reduce(
                out=partials[:, COL_CORR:COL_CORR + 1], in_=sc6[:],
                axis=AX.X, op=ALU.add,
            )

            # --- CIoU box loss at positive cells ---
            sx, sy, pw, ph = t6("sx"), t6("sy"), t6("pw"), t6("ph")
            nc.scalar.activation(sx[:], pos3[:, :, 0], AF.Exp, scale=-1.0)
            nc.vector.tensor_scalar_add(sx[:], sx[:], 1.0)
            nc.vector.reciprocal(sx[:], sx[:])                       # sigmoid(x0)
            nc.scalar.activation(sy[:], pos3[:, :, 1], AF.Exp, scale=-1.0)
            nc.vector.tensor_scalar_add(sy[:], sy[:], 1.0)
            nc.vector.reciprocal(sy[:], sy[:])                       # sigmoid(x1)
            nc.scalar.activation(pw[:], pos3[:, :, 2], AF.Exp)
            nc.scalar.activation(ph[:], pos3[:, :, 3], AF.Exp)
            px, py = t6("px"), t6("py")
            nc.vector.scalar_tensor_tensor(
                out=px[:], in0=sx[:], scalar=8.0, in1=cx8, op0=ALU.mult, op1=ALU.add)
            nc.vector.scalar_tensor_tensor(
                out=py[:], in0=sy[:], scalar=8.0, in1=cy8, op0=ALU.mult, op1=ALU.add)
            nc.vector.tensor_mul(pw[:], pw[:], awpx)
            nc.vector.tensor_mul(ph[:], ph[:], ahpx)
            px1, px2, py1, py2 = t6("px1"), t6("px2"), t6("py1"), t6("py2")
            hw, hh = t6("hw"), t6("hh")
            nc.vector.tensor_scalar_mul(hw[:], pw[:], 0.5)
            nc.vector.tensor_scalar_mul(hh[:], ph[:], 0.5)
            nc.vector.tensor_sub(px1[:], px[:], hw[:])
            nc.vector.tensor_add(px2[:], px[:], hw[:])
            nc.vector.tensor_sub(py1[:], py[:], hh[:])
            nc.vector.tensor_add(py2[:], py[:], hh[:])
            a6, b6, iw, ih = t6("a6"), t6("b6"), t6("iw"), t6("ih")
            nc.vector.tensor_tensor(out=a6[:], in0=px2[:], in1=gx2, op=ALU.min)
            nc.vector.tensor_tensor(out=b6[:], in0=px1[:], in1=gx1, op=ALU.max)
            nc.vector.tensor_sub(iw[:], a6[:], b6[:])
            nc.vector.tensor_scalar_max(iw[:], iw[:], 0.0)
            nc.vector.tensor_tensor(out=a6[:], in0=py2[:], in1=gy2, op=ALU.min)
            nc.vector.tensor_tensor(out=b6[:], in0=py1[:], in1=gy1, op=ALU.max)
            nc.vector.tensor_sub(ih[:], a6[:], b6[:])
            nc.vector.tensor_scalar_max(ih[:], ih[:], 0.0)
            inter = t6("inter")
            nc.vector.tensor_mul(inter[:], iw[:], ih[:])
            # union = relu(px2-px1)*relu(py2-py1) + areag - inter + EPS
            ap_, bp_ = t6("ap_"), t6("bp_")
            nc.vector.tensor_sub(ap_[:], px2[:], px1[:])
            nc.vector.tensor_scalar_max(ap_[:], ap_[:], 0.0)
            nc.vector.tensor_sub(bp_[:], py2[:], py1[:])
            nc.vector.tensor_scalar_max(bp_[:], bp_[:], 0.0)
            union = t6("union")
            nc.vector.tensor_mul(union[:], ap_[:], bp_[:])
            nc.vector.tensor_add(union[:], union[:], areag)
            nc.vector.tensor_sub(union[:], union[:], inter[:])
            nc.vector.tensor_scalar_add(union[:], union[:], float(EPS))
            iou = t6("iou")
            nc.vector.reciprocal(iou[:], union[:])
            nc.vector.tensor_mul(iou[:], inter[:], iou[:])
            # enclosing box diag^2
            cw, chv = t6("cw"), t6("chv")
            nc.vector.tensor_tensor(out=a6[:], in0=px2[:], in1=gx2, op=ALU.max)
            nc.vector.tensor_tensor(out=b6[:], in0=px1[:], in1=gx1, op=ALU.min)
            nc.vector.tensor_sub(cw[:], a6[:], b6[:])
            nc.vector.tensor_scalar_max(cw[:], cw[:], 0.0)
            nc.vector.tensor_tensor(out=a6[:], in0=py2[:], in1=gy2, op=ALU.max)
            nc.vector.tensor_tensor(out=b6[:], in0=py1[:], in1=gy1, op=ALU.min)
            nc.vector.tensor_sub(chv[:], a6[:], b6[:])
            nc.vector.tensor_scalar_max(chv[:], chv[:], 0.0)
            c2 = t6("c2")
            nc.vector.tensor_mul(cw[:], cw[:], cw[:])
            nc.vector.tensor_mul(chv[:], chv[:], chv[:])
            nc.vector.tensor_add(c2[:], cw[:], chv[:])
            nc.vector.tensor_scalar_add(c2[:], c2[:], float(EPS))
            rho2 = t6("rho2")
            nc.vector.tensor_tensor(out=a6[:], in0=px[:], in1=gxm, op=ALU.subtract)
            nc.vector.tensor_mul(a6[:], a6[:], a6[:])
            nc.vector.tensor_tensor(out=b6[:], in0=py[:], in1=gym, op=ALU.subtract)
            nc.vector.tensor_mul(b6[:], b6[:], b6[:])
            nc.vector.tensor_add(rho2[:], a6[:], b6[:])
            # atan(pw/(ph+EPS)) via polynomial (no trig table)
            q, qi, z, z2 = t6("q"), t6("qi"), t6("z"), t6("z2")
            nc.vector.tensor_scalar_add(q[:], ph[:], float(EPS))
            nc.vector.reciprocal(q[:], q[:])
            nc.vector.tensor_mul(q[:], pw[:], q[:])                  # q > 0
            nc.vector.reciprocal(qi[:], q[:])
            nc.vector.tensor_tensor(out=z[:], in0=q[:], in1=qi[:], op=ALU.min)
            nc.vector.tensor_mul(z2[:], z[:], z[:])
            acc = t6("acc")
            nc.vector.tensor_scalar(
                out=acc[:], in0=z2[:], scalar1=float(ATAN_C[6]),
                scalar2=float(ATAN_C[5]), op0=ALU.mult, op1=ALU.add)
            for k in (4, 3, 2, 1, 0):
                nc.vector.tensor_mul(acc[:], acc[:], z2[:])
                nc.vector.tensor_scalar_add(acc[:], acc[:], float(ATAN_C[k]))
            nc.vector.tensor_mul(acc[:], acc[:], z[:])               # atan(z)
            flag = t6("flag")
            nc.vector.tensor_scalar(
                out=flag[:], in0=q[:], scalar1=1.0, scalar2=None, op0=ALU.is_gt)
            fw = t6("fw")
            nc.vector.tensor_scalar(
                out=fw[:], in0=acc[:], scalar1=-2.0,
                scalar2=float(np.pi / 2), op0=ALU.mult, op1=ALU.add)
            nc.vector.tensor_mul(fw[:], fw[:], flag[:])
            nc.vector.tensor_add(acc[:], acc[:], fw[:])              # atan(q)
            vv = t6("vv")
            nc.vector.tensor_tensor(out=vv[:], in0=atg, in1=acc[:], op=ALU.subtract)
            nc.vector.tensor_mul(vv[:], vv[:], vv[:])
            nc.vector.tensor_scalar_mul(vv[:], vv[:], float(np.float32(4.0) / PI2))
            # alpha = v / (1 - iou + v + EPS)
            den = t6("den")
            nc.vector.scalar_tensor_tensor(
                out=den[:], in0=iou[:], scalar=-1.0, in1=vv[:],
                op0=ALU.mult, op1=ALU.add)
            nc.vector.tensor_scalar_add(den[:], den[:], float(1.0 + float(EPS)))
            nc.vector.reciprocal(den[:], den[:])
            nc.vector.tensor_mul(den[:], vv[:], den[:])              # alpha
            nc.vector.tensor_mul(den[:], den[:], vv[:])              # alpha*v
            # loss = 1 - iou + rho2/c2 + alpha*v
            nc.vector.reciprocal(c2[:], c2[:])
            nc.vector.tensor_mul(rho2[:], rho2[:], c2[:])
            nc.vector.tensor_add(den[:], den[:], rho2[:])
            nc.vector.tensor_sub(den[:], den[:], iou[:])
            nc.vector.tensor_scalar_add(den[:], den[:], 1.0)
            bsc = t6("bsc")
            nc.vector.tensor_mul(bsc[:], den[:], valid)
            nc.vector.tensor_reduce(
                out=partials[:, COL_BOX:COL_BOX + 1], in_=bsc[:],
                axis=AX.X, op=ALU.add,
            )

            # --- weighted focal class loss at positive cells ---
            NCL = NGRP * C
            xc = pos3[:, :, 5:]                                      # [128,6,80]

            def tcl(name):
                return smallp.tile([128, NCL], f32, name=name)

            ecl, scl, lcl = tcl("ecl"), tcl("scl"), tcl("lcl")
            ucl, fcl, sccl = tcl("ucl"), tcl("fcl"), tcl("sccl")
            e3 = ecl[:].rearrange("p (g c) -> p g c", c=C)
            nc.scalar.activation(e3, xc, AF.Exp)                     # e^x
            nc.vector.tensor_scalar_add(scl[:], ecl[:], 1.0)
            nc.vector.reciprocal(scl[:], scl[:])                     # 1 - sigmoid
            nc.vector.tensor_scalar(
                out=scl[:], in0=scl[:], scalar1=-1.0, scalar2=1.0,
                op0=ALU.mult, op1=ALU.add)                           # sigmoid
            nc.scalar.activation(lcl[:], ecl[:], AF.Ln, bias=1.0)    # softplus
            nc.vector.tensor_scalar_min(lcl[:], lcl[:], 88.0)
            nc.vector.tensor_mul(ucl[:], scl[:], tcls_t[:])          # s*t
            nc.vector.scalar_tensor_tensor(
                out=ucl[:], in0=ucl[:], scalar=-2.0, in1=scl[:],
                op0=ALU.mult, op1=ALU.add)                           # s - 2st
            nc.vector.tensor_add(ucl[:], ucl[:], tcls_t[:])          # u
            nc.vector.tensor_scalar_max(ucl[:], ucl[:], 1e-38)
            nc.scalar.activation(ucl[:], ucl[:], AF.Ln)
            nc.scalar.activation(ucl[:], ucl[:], AF.Exp, scale=1.5)  # u^1.5
            f3 = fcl[:].rearrange("p (g c) -> p g c", c=C)
            nc.vector.tensor_tensor(out=f3, in0=xc, in1=tcls_t[:].rearrange(
                "p (g c) -> p g c", c=C), op=ALU.mult)               # x*t
            nc.vector.tensor_sub(fcl[:], lcl[:], fcl[:])             # bce
            nc.vector.tensor_mul(fcl[:], ucl[:], fcl[:])
            nc.vector.tensor_mul(sccl[:], fcl[:], wq_t[:])
            nc.vector.tensor_reduce(
                out=partials[:, COL_CLS:COL_CLS + 1], in_=sccl[:],
                axis=AX.X, op=ALU.add,
            )


            if mode == "full":
                pt = p[:].rearrange("(t p k) c -> t p (k c)", t=NT, p=128)
                for t in range(NT):
                    xt = streamp.tile([128, KC * CH], f32, tag="xt", name="xt")
                    nc.sync.dma_start(out=xt[:], in_=pt[t])
                    ch4 = xt[:].rearrange("p (k c) -> p k c", c=CH)[:, :, 4]
                    obj_dense(ch4, KC, t)
            elif mode == "pair":
                # one descriptor spans ch4 of two adjacent cells (86 floats):
                # halves descriptor count; engines move 344B instead of 2x4B
                NPAIR = NCELL // 2           # 38400
                NTP = 6
                KP = NPAIR // (NTP * 128)    # 50 pairs/partition/tile
                for t in range(NTP):
                    xt = streamp.tile([128, KP * 86], f32, tag="xp", name="xp")
                    src = bass.AP(
                        tensor=p[:].tensor,
                        offset=4 + t * (128 * KP) * 170,
                        ap=[[170 * KP, 128], [170, KP], [1, 86]],
                    )
                    eng = nc.sync if t % 2 == 0 else nc.scalar
                    eng.dma_start(out=xt[:].rearrange(
                        "q (k c) -> q k c", c=86), in_=src)
                    ch4 = xt[:].rearrange("q (k c) -> q k c", c=86)[:, :, 0:86:85]
                    obj_dense(ch4, [KP, 2], t)
            elif mode == "strided":
                ps4 = p[:].rearrange("(t p k) c -> t p k c", t=NTS, p=128)
                for t in range(NTS):
                    xt = streamp.tile([128, KS], f32, tag="xs", name="xs")
                    nc.sync.dma_start(out=xt[:], in_=ps4[t, :, :, 4])
                    obj_dense(xt[:], KS, t)
            elif mode == "strided3":
                # N=1 descriptors (engine-cost optimal), both HWDGE rings,
                # deep buffering so all DMAs stay in flight
                NT3 = 8
                K3 = NCELL // (NT3 * 128)
                ps8 = p[:].rearrange("(t p k) c -> t p k c", t=NT3, p=128)
                for t in range(NT3):
                    xt = streamp.tile([128, K3], f32, tag="xs3", name="xs3",
                                      bufs=NT3)
                    eng = nc.sync if t % 2 == 0 else nc.scalar
                    eng.dma_start(out=xt[:], in_=ps8[t, :, :, 4])
                    obj_dense(xt[:], K3, t)
            else:  # strided2: split ch4 extraction over both HWDGE rings
                NT2 = 8
                K2 = NCELL // (NT2 * 128)
                ps8 = p[:].rearrange("(t p k) c -> t p k c", t=NT2, p=128)
                for t in range(NT2):
                    xt = streamp.tile([128, K2], f32, tag="xs2", name="xs2")
                    eng = nc.sync if t % 2 == 0 else nc.scalar
                    eng.dma_start(out=xt[:], in_=ps8[t, :, :, 4])
                    obj_dense(xt[:], K2, t)

            # ---------------- cross-partition reduce + store ----------------
            ones = smallp.tile([128, 1], f32)
            nc.vector.memset(ones[:], 1.0)
            ps = psump.tile([1, NCOL], f32)
            nc.tensor.matmul(out=ps[:], lhsT=ones[:], rhs=partials[:],
                             start=True, stop=True)
            res = smallp.tile([1, NCOL], f32)
            nc.vector.tensor_copy(out=res[:], in_=ps[:])
            nc.sync.dma_start(out=outp[:], in_=res[:])

    _split_multi_waits(nc)
    return nc


def _assign_targets_host(labels, label_mask, cls_weight):
    """Replicate reference.assign_targets scatter on host; returns per-core
    device aux inputs and global n_pos."""
    labels = np.asarray(labels, dtype=np.float32)
    mask = np.asarray(label_mask).astype(bool)
    cw = np.asarray(cls_weight, dtype=np.float32)

    gcls = labels[..., 0].astype(np.int32)                      # [B, M]
    gx = labels[..., 1] * IMG
    gy = labels[..., 2] * IMG
    gw = labels[..., 3] * IMG
    gh = labels[..., 4] * IMG
    gi = np.clip(gx / STRIDE, np.float32(0.0), np.float32(W - 0.001)).astype(np.int32)
    gj = np.clip(gy / STRIDE, np.float32(0.0), np.float32(H - 0.001)).astype(np.int32)
    gtw, gth = gw / STRIDE, gh / STRIDE
    ag = ANCHORS / STRIDE                                       # [3, 2]
    inter = np.minimum(gtw[..., None], ag[:, 0]) * np.minimum(gth[..., None], ag[:, 1])
    union = gtw[..., None] * gth[..., None] + ag[:, 0] * ag[:, 1] - inter + np.float32(1e-9)
    best_a = np.argmax(inter / union, axis=-1).astype(np.int32)  # [B, M]

    offs = [(di, dj) for di in (-1, 0, 1) for dj in (-1, 0, 1)]
    # sequential scatter with last-write-wins box, accumulating class set
    targets = {}  # (b, a, j, i) -> [set(cls), (bx, by, bw, bh)]
    for b in range(B):
        for m in range(M):
            if not mask[b, m]:
                continue
            a = int(best_a[b, m])
            c = int(gcls[b, m])
            box = (gx[b, m], gy[b, m], gw[b, m], gh[b, m])
            for di, dj in offs:
                i = min(max(int(gi[b, m]) + di, 0), W - 1)
                j = min(max(int(gj[b, m]) + dj, 0), H - 1)
                e = targets.setdefault((b, a, j, i), [set(), None])
                e[0].add(c)
                e[1] = box
    n_pos = max(len(targets), 1)

    idx_all = np.zeros((NCORES, 128, NGRP), dtype=np.int32)
    meta_all = np.zeros((NCORES, 128, NGRP * NMETA), dtype=np.float32)
    tcls_all = np.zeros((NCORES, 128, NGRP * C), dtype=np.float32)
    wq_all = np.zeros((NCORES, 128, NGRP * C), dtype=np.float32)
    slot_ctr = [0] * NCORES
    for (b, a, j, i), (clsset, box) in targets.items():
        core = b // BL
        s = slot_ctr[core]
        slot_ctr[core] += 1
        assert s < NPOS, "positive-cell capacity exceeded"
        p_, g_ = s % 128, s // 128
        bloc = b - core * BL
        idx_all[core, p_, g_] = ((bloc * NA + a) * H + j) * W + i
        bx, by, bw, bh = box
        gx1 = bx - bw * np.float32(0.5)
        gx2 = bx + bw * np.float32(0.5)
        gy1 = by - bh * np.float32(0.5)
        gy2 = by + bh * np.float32(0.5)
        areag = max(gx2 - gx1, np.float32(0.0)) * max(gy2 - gy1, np.float32(0.0))
        atg = np.float32(np.arctan(bw / (bh + EPS)))
        mslot = np.array(
            [1.0, i * 8.0, j * 8.0, ANCHORS[a, 0], ANCHORS[a, 1],
             bx, by, gx1, gx2, gy1, gy2, areag, atg, 0.0, 0.0, 0.0],
            dtype=np.float32,
        )
        meta_all[core, p_, g_ * NMETA:(g_ + 1) * NMETA] = mslot
        for c in clsset:
            tcls_all[core, p_, g_ * C + c] = 1.0
        wq_all[core, p_, g_ * C:(g_ + 1) * C] = np.float32(0.25) * cw
    return idx_all, meta_all, tcls_all, wq_all, n_pos


def kernel(p_raw, labels, label_mask, cls_weight):
    global LAST_RESULT
    p_raw = np.ascontiguousarray(np.asarray(p_raw, dtype=np.float32))
    idx_all, meta_all, tcls_all, wq_all, n_pos = _assign_targets_host(
        labels, label_mask, cls_weight
    )

    if MODE not in _BUILD_CACHE:
        _BUILD_CACHE[MODE] = _build(MODE)
    nc = _BUILD_CACHE[MODE]

    shards = p_raw.reshape(NCORES, NCELL, CH)
    in_maps = []
    for c in range(NCORES):
        in_maps.append({
            "p": shards[c],
            "idx": idx_all[c],
            "meta": meta_all[c],
            "tcls": tcls_all[c],
            "wq": wq_all[c],
        })

    r = run_bass_kernel_spmd(
        nc, in_maps, core_ids=list(range(NCORES)), trace=TRACE, **TRACE_KW
    )
    LAST_RESULT = r

    outs = np.stack([np.asarray(r.results[c]["out"][0]) for c in range(NCORES)])
    sums = outs.astype(np.float64).sum(axis=0)
    s_dense = sums[:COL_CORR].sum()
    l_obj = 0.25 * (s_dense + sums[COL_CORR]) / float(B * NA * H * W)
    l_box = sums[COL_BOX] / n_pos
    l_cls = sums[COL_CLS] / (n_pos * C)
    total = 7.5 * l_box + 1.0 * l_obj + 0.5 * l_cls
    return np.float32(total)
